# revision 24
# baseline (speedup 1.0000x reference)
"""Trainium2 Bass kernel for nn_Block_26628797235524 (Mamba-style cross-scan SSM block).

Sharding: batch B=8 -> one batch element per NeuronCore (8 cores, SPMD, no
collectives). Each core runs the full block for its batch element:
  in_proj -> conv(1x1x1)+silu -> dual-order selective scan (K=2, DIN=256,
  DST=16) -> combine -> layernorm -> gate -> out_proj.

v2: fp16 hot path (DVE 2-byte fast modes, 1-cyc/row PE matmuls), fused
Silu/Softplus activations, Ds skip-connection folded into the PSUM
accumulation via a diagonal matmul, xs/z kept resident in SBUF (no DRAM
round-trips), fp16 B/C broadcasts. Scan runs as tensor_tensor_scan over
128-channel x 1024-step slabs chained via the last column; state is fp32
inside the scan instruction so fp16 operands only quantize the readout.

kernel(**inputs) takes the FULL unsharded inputs and returns the FULL output.
"""

import os
import sys
from contextlib import ExitStack

import numpy as np

_RL = "/opt/trn_rl_repo"
if os.path.isdir(_RL) and _RL not in sys.path:
    sys.path.insert(0, _RL)

import concourse.bass as bass
import concourse.bacc as bacc
import concourse.tile as tile
from concourse import mybir
from concourse.bass_utils import run_bass_kernel_spmd

# Problem sizes (hardcoded per the task contract).
B, T, H, W, DIM = 8, 16, 16, 16, 128
DIN, DST, DTR, KG = 256, 16, 8, 2
L = T * H * W          # 4096
P = 128                # partitions
DH = DIN // P          # 2 d-half tiles per direction
LC = 1024              # scan slab length
NQ = L // LC           # 4
NCORES = 8

F32 = mybir.dt.float32
F16 = mybir.dt.float16
AF = mybir.ActivationFunctionType
ALU = mybir.AluOpType
MM_F = 512             # matmul free-dim chunk (one PSUM bank)
NMM = L // MM_F        # 8 chunks over L

# Engine split knobs for the scan inner loop (tuned from traces).
# Scans are DVE-only (backend rejects Pool TensorScalarPtr); gpsimd takes the
# C-side multiplies and a slice of the B-side ones to balance.
TMP_ON_GPSIMD = lambda n, i: True
XIN_ON_GPSIMD = lambda n, i: n == 15


def _declare_drams(nc):
    d = {}

    def inp(name, shape, dt=F16):
        d[name] = nc.dram_tensor(name, list(shape), dt, kind="ExternalInput")

    inp("xT", (P, L))                       # per-core batch slice, channel-major
    inp("w_in", (P, 4 * P))                 # in_proj_w.T
    inp("conv_sc", (DH, P, 1), F32)
    inp("conv_bi", (DH, P, 1), F32)
    inp("w_xproj", (KG, DH, P, 40))         # x_proj_w[k].T in 2 pi-chunks
    inp("w_dt", (KG, DTR, DIN))             # dt_w[k].T
    inp("dt_bias", (KG, DH, P, 1), F32)
    inp("a_mat", (KG, DH, P, DST), F32)     # A = -exp(A_logs)
    inp("ds_diag", (KG, DH, P, P))          # diag(Ds) per (k, half)
    inp("lnw", (DH, P, 1), F32)
    inp("lnb", (DH, P, 1), F32)
    inp("w_out", (DH, P, P))                # out_proj_w.T in 2 pi-chunks
    inp("ident", (P, P))                    # identity: PE accumulate matmuls
    d["bc_dram"] = nc.dram_tensor("bc_dram", [KG, 2 * DST, L], F16)  # B/C rows
    d["outT"] = nc.dram_tensor("outT", [P, L], F32, kind="ExternalOutput")
    return d


def _body(tc, d):
    nc = tc.nc
    with ExitStack() as ctx:
        const = ctx.enter_context(tc.tile_pool(name="const", bufs=1))

        # ---- constants ----
        w_in = const.tile([P, 4 * P], F16, tag="w_in", name="w_in")
        nc.sync.dma_start(w_in[:], d["w_in"][:])
        conv_sc = [const.tile([P, 1], F32, tag=f"csc{i}", name=f"csc{i}") for i in range(DH)]
        conv_bi = [const.tile([P, 1], F32, tag=f"cbi{i}", name=f"cbi{i}") for i in range(DH)]
        for i in range(DH):
            nc.sync.dma_start(conv_sc[i][:], d["conv_sc"][i])
            nc.sync.dma_start(conv_bi[i][:], d["conv_bi"][i])
        w_xproj = [[const.tile([P, 40], F16, tag=f"wxp{k}{i}", name=f"wxp{k}{i}") for i in range(DH)]
                   for k in range(KG)]
        w_dt = [const.tile([DTR, DIN], F16, tag=f"wdt{k}", name=f"wdt{k}") for k in range(KG)]
        dt_bias = [[const.tile([P, 1], F32, tag=f"dtb{k}{i}", name=f"dtb{k}{i}") for i in range(DH)]
                   for k in range(KG)]
        a_mat = [[const.tile([P, DST], F32, tag=f"am{k}{i}", name=f"am{k}{i}") for i in range(DH)]
                 for k in range(KG)]
        ds_diag = [[const.tile([P, P], F16, tag=f"dsd{k}{i}", name=f"dsd{k}{i}") for i in range(DH)]
                   for k in range(KG)]
        for k in range(KG):
            nc.sync.dma_start(w_dt[k][:], d["w_dt"][k])
            for i in range(DH):
                nc.sync.dma_start(w_xproj[k][i][:], d["w_xproj"][k, i])
                nc.sync.dma_start(dt_bias[k][i][:], d["dt_bias"][k, i])
                nc.sync.dma_start(a_mat[k][i][:], d["a_mat"][k, i])
                nc.sync.dma_start(ds_diag[k][i][:], d["ds_diag"][k, i])
        lnw = [const.tile([P, 1], F32, tag=f"lnw{i}", name=f"lnw{i}") for i in range(DH)]
        lnb = [const.tile([P, 1], F32, tag=f"lnb{i}", name=f"lnb{i}") for i in range(DH)]
        w_out = [const.tile([P, P], F16, tag=f"wo{i}", name=f"wo{i}") for i in range(DH)]
        for i in range(DH):
            nc.sync.dma_start(lnw[i][:], d["lnw"][i])
            nc.sync.dma_start(lnb[i][:], d["lnb"][i])
            nc.sync.dma_start(w_out[i][:], d["w_out"][i])
        # 1/DIN-scaled ones column: the LN mean/second-moment contraction.
        oneN_col = const.tile([P, 1], F16, tag="oneN_col", name="oneN_col")
        nc.vector.memset(oneN_col[:], 1.0 / DIN)
        ones_row = const.tile([1, P], F16, tag="ones_row", name="ones_row")
        nc.vector.memset(ones_row[:], 1.0)
        ident = const.tile([P, P], F16, tag="ident", name="ident")
        nc.sync.dma_start(ident[:], d["ident"][:])
        eps = const.tile([1, 1], F32, tag="eps", name="eps")
        nc.vector.memset(eps[:], 1e-5)

        # ---- persistent activations (fp16 [P, L] = 8KB/partition each) ----
        main = ctx.enter_context(tc.tile_pool(name="main", bufs=1))
        xs = [[main.tile([P, L], F16, tag=f"xs{k}{i}", name=f"xs{k}{i}")
               for i in range(DH)] for k in range(KG)]
        z_sb = [main.tile([P, L], F16, tag=f"z{i}", name=f"z{i}") for i in range(DH)]
        y_k = [[main.tile([P, L], F16, tag=f"y{k}{i}", name=f"y{k}{i}")
                for i in range(DH)] for k in range(KG)]

        # ========== Phase 1: in_proj -> conv+silu -> scan orderings ==========
        with tc.tile_pool(name="p1", bufs=1) as p1pool, \
             tc.tile_pool(name="p1ps", bufs=4, space=bass.MemorySpace.PSUM) as p1ps:
            xT = p1pool.tile([P, L], F16, tag="xT", name="xT")
            nc.sync.dma_start(xT[:], d["xT"][:])
            for po in range(4):
                for c in range(NMM):
                    cs = slice(c * MM_F, (c + 1) * MM_F)
                    pt = p1ps.tile([P, MM_F], F32, tag="mm", name="mm")
                    nc.tensor.matmul(pt[:], w_in[:, po * P:(po + 1) * P],
                                     xT[:, cs], start=True, stop=True)
                    if po < DH:
                        # xh = silu(xz*conv_w + conv_b), written twice:
                        # (h w t) order for k=0 and reversed (t h w) for k=1.
                        src3 = pt[:].rearrange("p (t hw) -> p t hw", t=2, hw=H * W)
                        dst3 = xs[0][po][:].rearrange(
                            "p (hw t) -> p t hw", hw=H * W, t=T)[:, 2 * c:2 * c + 2, :]
                        nc.scalar.activation(dst3, src3, AF.Silu,
                                             bias=conv_bi[po][:], scale=conv_sc[po][:])
                        rev = xs[1][po][:, ::-1]
                        nc.scalar.activation(rev[:, cs], pt[:], AF.Silu,
                                             bias=conv_bi[po][:], scale=conv_sc[po][:])
                    else:
                        nc.scalar.activation(z_sb[po - DH][:, cs], pt[:], AF.Silu)

        # ================= Per-direction pipeline =================
        for k in range(KG):
            with tc.tile_pool(name=f"kp{k}", bufs=1) as kpool:
                delta = [kpool.tile([P, L], F16, tag=f"delta{i}", name=f"delta{i}")
                         for i in range(DH)]
                dU = [kpool.tile([P, L], F16, tag=f"dU{i}", name=f"dU{i}")
                      for i in range(DH)]
                # ---- Phase 3: x_dbl -> (dts, B, C); delta = softplus ----
                with tc.tile_pool(name=f"kd{k}", bufs=1) as kdpool, \
                     tc.tile_pool(name=f"kps{k}", bufs=1,
                                  space=bass.MemorySpace.PSUM) as kps:
                    xdbl = kdpool.tile([DTR, L], F16, tag="xdbl", name="xdbl")
                    for c in range(NMM):
                        cs = slice(c * MM_F, (c + 1) * MM_F)
                        ptA = kps.tile([DTR, MM_F], F32, tag="mmA", name="mmA")
                        nc.tensor.matmul(ptA[:], w_xproj[k][0][:, 0:DTR],
                                         xs[k][0][:, cs], start=True, stop=False)
                        nc.tensor.matmul(ptA[:], w_xproj[k][1][:, 0:DTR],
                                         xs[k][1][:, cs], start=False, stop=True)
                        nc.scalar.activation(xdbl[:, cs], ptA[:], AF.Copy)
                        ptB = kps.tile([2 * DST, MM_F], F32, tag="mmB", name="mmB")
                        nc.tensor.matmul(ptB[:], w_xproj[k][0][:, DTR:40],
                                         xs[k][0][:, cs], start=True, stop=False)
                        nc.tensor.matmul(ptB[:], w_xproj[k][1][:, DTR:40],
                                         xs[k][1][:, cs], start=False, stop=True)
                        bcs = kdpool.tile([2 * DST, MM_F], F16, tag="bcs",
                                          name="bcs", bufs=3)
                        nc.scalar.activation(bcs[:], ptB[:], AF.Copy)
                        nc.sync.dma_start(d["bc_dram"][k, :, cs], bcs[:])
                    for i in range(DH):
                        for c in range(NMM):
                            cs = slice(c * MM_F, (c + 1) * MM_F)
                            pt2 = kps.tile([P, MM_F], F32, tag="mmd", name="mmd")
                            nc.tensor.matmul(pt2[:], w_dt[k][:, i * P:(i + 1) * P],
                                             xdbl[:, cs], start=True, stop=True)
                            # softplus(x + b) = ln(1 + exp(x + b))
                            ed = kdpool.tile([P, MM_F], F16, tag="ed",
                                             name="ed", bufs=3)
                            nc.scalar.activation(ed[:], pt2[:], AF.Exp,
                                                 bias=dt_bias[k][i][:])
                            nc.scalar.activation(delta[i][:, cs], ed[:],
                                                 AF.Ln, bias=1.0)
                    for i in range(DH):
                        nc.vector.tensor_tensor(dU[i][:], delta[i][:], xs[k][i][:],
                                                ALU.mult)

                # ---- Phase 4: selective scan (q -> n -> i) ----
                states = [kpool.tile([P, DST], F16, tag=f"st{i}", name=f"st{i}")
                          for i in range(DH)]
                with tc.tile_pool(name=f"sc{k}", bufs=2) as work, \
                     tc.tile_pool(name=f"bc{k}", bufs=3) as bcp, \
                     tc.tile_pool(name=f"scps{k}", bufs=2,
                                  space=bass.MemorySpace.PSUM) as scps:
                    for q in range(NQ):
                        sl = slice(q * LC, (q + 1) * LC)
                        y_ps = [scps.tile([P, LC], F32, tag=f"yps{i}",
                                          name=f"yps{i}") for i in range(DH)]
                        for i in range(DH):
                            for hb in range(LC // MM_F):
                                ps_ = slice(hb * MM_F, (hb + 1) * MM_F)
                                gs = slice(q * LC + hb * MM_F,
                                           q * LC + (hb + 1) * MM_F)
                                nc.tensor.matmul(y_ps[i][:, ps_], ds_diag[k][i][:],
                                                 xs[k][i][:, gs],
                                                 start=True, stop=False)
                        for n in range(DST):
                            brep = bcp.tile([P, LC], F16, tag="brep", name="brep")
                            nc.sync.dma_start(
                                brep[:],
                                d["bc_dram"][k, n:n + 1, sl].partition_broadcast(P))
                            crep = bcp.tile([P, LC], F16, tag="crep", name="crep")
                            nc.sync.dma_start(
                                crep[:],
                                d["bc_dram"][k, DST + n:DST + n + 1,
                                             sl].partition_broadcast(P))
                            for i in range(DH):
                                dA = work.tile([P, LC], F16, tag=f"dA{i}",
                                               name=f"dA{i}")
                                nc.scalar.activation(dA[:], delta[i][:, sl], AF.Exp,
                                                     scale=a_mat[k][i][:, n:n + 1])
                                xin = work.tile([P, LC], F16, tag="xin", name="xin")
                                xeng = nc.gpsimd if XIN_ON_GPSIMD(n, i) else nc.vector
                                xeng.tensor_tensor(xin[:], dU[i][:, sl],
                                                   brep[:], ALU.mult)
                                h = work.tile([P, LC], F16, tag=f"h{i}",
                                              name=f"h{i}")
                                init = 0.0 if q == 0 else states[i][:, n:n + 1]
                                nc.vector.tensor_tensor_scan(
                                    h[:], dA[:], xin[:], init, ALU.mult, ALU.add)
                                if q < NQ - 1:
                                    nc.vector.tensor_copy(states[i][:, n:n + 1],
                                                          h[:, LC - 1:LC])
                                tmp = work.tile([P, LC], F16, tag="tmp", name="tmp")
                                eng = nc.gpsimd if TMP_ON_GPSIMD(n, i) else nc.vector
                                eng.tensor_tensor(tmp[:], crep[:], h[:], ALU.mult)
                                for hb in range(LC // MM_F):
                                    ps_ = slice(hb * MM_F, (hb + 1) * MM_F)
                                    nc.tensor.matmul(y_ps[i][:, ps_], ident[:],
                                                     tmp[:, ps_],
                                                     start=False,
                                                     stop=(n == DST - 1))
                        for i in range(DH):
                            nc.scalar.activation(y_k[k][i][:, sl], y_ps[i][:],
                                                 AF.Copy)

        # ================= Phase 5-7: combine, LN, gate, out_proj =================
        with tc.tile_pool(name="fin", bufs=1) as fin:
            ysum = [fin.tile([P, L], F16, tag=f"ys{i}", name=f"ys{i}")
                    for i in range(DH)]
            for i in range(DH):
                # y = reorder(y_fwd) + flip(y_rvs), in (t, hw) natural order
                src0 = y_k[0][i][:].rearrange("p (hw t) -> p t hw", hw=H * W, t=T)
                src1 = y_k[1][i][:, ::-1].rearrange("p (t hw) -> p t hw", t=T, hw=H * W)
                dst = ysum[i][:].rearrange("p (t hw) -> p t hw", t=T, hw=H * W)
                nc.vector.tensor_tensor(dst, src0, src1, ALU.add)

            # LN stats over DIN (partition reduce via PE 1/DIN-ones contraction)
            mu16 = fin.tile([1, L], F16, tag="mu16", name="mu16")
            m2_16 = fin.tile([1, L], F16, tag="m2_16", name="m2_16")
            sq16 = fin.tile([1, L], F16, tag="sq16", name="sq16")
            rs16 = fin.tile([1, L], F16, tag="rs16", name="rs16")
            with tc.tile_pool(name="fps1", bufs=4,
                              space=bass.MemorySpace.PSUM) as fps1:
                for c in range(NMM):
                    cs = slice(c * MM_F, (c + 1) * MM_F)
                    pmu = fps1.tile([1, MM_F], F32, tag="pmu", name="pmu")
                    nc.tensor.matmul(pmu[:], oneN_col[:], ysum[0][:, cs],
                                     start=True, stop=False)
                    nc.tensor.matmul(pmu[:], oneN_col[:], ysum[1][:, cs],
                                     start=False, stop=True)
                    nc.scalar.activation(mu16[:, cs], pmu[:], AF.Copy)
                    psq = fps1.tile([1, MM_F], F32, tag="psq", name="psq")
                    for i in range(DH):
                        ysq = fin.tile([P, MM_F], F16, tag="ysq", name="ysq",
                                       bufs=2)
                        nc.scalar.activation(ysq[:], ysum[i][:, cs], AF.Square)
                        nc.tensor.matmul(psq[:], oneN_col[:], ysq[:],
                                         start=(i == 0), stop=(i == DH - 1))
                    nc.scalar.activation(m2_16[:, cs], psq[:], AF.Copy)
            # var = E[y^2] - mu^2 ; rstd = exp(-0.5*ln(var + eps))
            nc.vector.tensor_tensor(sq16[:], mu16[:], mu16[:], ALU.mult)
            nc.vector.tensor_tensor(m2_16[:], m2_16[:], sq16[:], ALU.subtract)
            nc.scalar.activation(sq16[:], m2_16[:], AF.Ln, bias=eps[:1, :])
            nc.scalar.activation(rs16[:], sq16[:], AF.Exp, scale=-0.5)

            # normalize + affine + gate + out_proj, chunked over L
            with tc.tile_pool(name="fch", bufs=2) as fch, \
                 tc.tile_pool(name="fps2", bufs=2,
                              space=bass.MemorySpace.PSUM) as fps2:
                for c in range(NMM):
                    cs = slice(c * MM_F, (c + 1) * MM_F)
                    pm = fps2.tile([P, MM_F], F32, tag="pm", name="pm")
                    nc.tensor.matmul(pm[:], ones_row[:], mu16[:, cs],
                                     start=True, stop=True)
                    mrep = fch.tile([P, MM_F], F16, tag="mrep", name="mrep")
                    nc.scalar.activation(mrep[:], pm[:], AF.Copy)
                    pr = fps2.tile([P, MM_F], F32, tag="pr", name="pr")
                    nc.tensor.matmul(pr[:], ones_row[:], rs16[:, cs],
                                     start=True, stop=True)
                    rrep = fch.tile([P, MM_F], F16, tag="rrep", name="rrep")
                    nc.scalar.activation(rrep[:], pr[:], AF.Copy)
                    g = []
                    for i in range(DH):
                        yc = fch.tile([P, MM_F], F16, tag="yc", name="yc")
                        nc.vector.tensor_tensor(yc[:], ysum[i][:, cs], mrep[:],
                                                ALU.subtract)
                        yn = fch.tile([P, MM_F], F16, tag="yn", name="yn")
                        nc.vector.tensor_tensor(yn[:], yc[:], rrep[:], ALU.mult)
                        ya = fch.tile([P, MM_F], F16, tag="ya", name="ya")
                        nc.scalar.activation(ya[:], yn[:], AF.Identity,
                                             bias=lnb[i][:], scale=lnw[i][:])
                        gi = fch.tile([P, MM_F], F16, tag=f"g{i}", name=f"g{i}")
                        nc.vector.tensor_tensor(gi[:], ya[:], z_sb[i][:, cs],
                                                ALU.mult)
                        g.append(gi)
                    po = fps2.tile([P, MM_F], F32, tag="pout", name="pout")
                    nc.tensor.matmul(po[:], w_out[0][:], g[0][:],
                                     start=True, stop=False)
                    nc.tensor.matmul(po[:], w_out[1][:], g[1][:],
                                     start=False, stop=True)
                    osb = fch.tile([P, MM_F], F32, tag="osb", name="osb")
                    nc.scalar.activation(osb[:], po[:], AF.Copy)
                    nc.sync.dma_start(d["outT"][:, cs], osb[:])


_CACHE = {}


def _get_program():
    if "nc" not in _CACHE:
        nc = bacc.Bacc("TRN2", target_bir_lowering=False, debug=False,
                       num_devices=NCORES)
        d = _declare_drams(nc)
        with tile.TileContext(nc) as tc:
            _body(tc, d)
        nc.compile()
        _CACHE["nc"] = nc
    return _CACHE["nc"]


def _host_weights(inputs):
    f32 = lambda a: np.ascontiguousarray(np.asarray(a, np.float32))
    f16 = lambda a: np.ascontiguousarray(np.asarray(a, np.float32).astype(np.float16))
    in_proj_w = f32(inputs["in_proj_w"])        # (512, 128)
    x_proj_w = f32(inputs["x_proj_w"])          # (2, 40, 256)
    dt_w = f32(inputs["dt_w"])                  # (2, 256, 8)
    dt_b = f32(inputs["dt_b"])                  # (2, 256)
    A_logs = f32(inputs["A_logs"])              # (512, 16)
    Ds = f32(inputs["Ds"])                      # (512,)
    ds_diag = np.zeros((KG, DH, P, P), np.float16)
    dsr = Ds.reshape(KG, DH, P)
    for k in range(KG):
        for i in range(DH):
            np.fill_diagonal(ds_diag[k, i], dsr[k, i].astype(np.float16))
    m = {
        "w_in": f16(in_proj_w.T),                                   # (128, 512)
        "conv_sc": f32(inputs["conv_w"]).reshape(DH, P, 1),
        "conv_bi": f32(inputs["conv_b"]).reshape(DH, P, 1),
        "w_xproj": f16(x_proj_w.transpose(0, 2, 1).reshape(KG, DH, P, 40)),
        "w_dt": f16(dt_w.transpose(0, 2, 1)),                       # (2, 8, 256)
        "dt_bias": f32(dt_b).reshape(KG, DH, P, 1),
        "a_mat": f32(-np.exp(A_logs)).reshape(KG, DH, P, DST),
        "ds_diag": ds_diag,
        "lnw": f32(inputs["ln_w"]).reshape(DH, P, 1),
        "lnb": f32(inputs["ln_b"]).reshape(DH, P, 1),
        "w_out": f16(f32(inputs["out_proj_w"]).T.reshape(DH, P, P)),
        "ident": np.eye(P, dtype=np.float16),
    }
    return m


def kernel(**inputs):
    x = np.ascontiguousarray(np.asarray(inputs["x"], np.float32))   # (8,16,16,16,128)
    shared = _host_weights(inputs)
    nc = _get_program()
    in_maps = []
    for b in range(NCORES):
        m = dict(shared)
        m["xT"] = np.ascontiguousarray(x[b].reshape(L, DIM).T).astype(np.float16)
        in_maps.append(m)
    trace = bool(int(os.environ.get("BASS_PROFILE", "0")))
    res = run_bass_kernel_spmd(nc, in_maps, list(range(NCORES)), trace=trace)
    _CACHE["last_result"] = res
    outs = [r["outT"] for r in res.results]
    out = np.stack([o.T.reshape(T, H, W, DIM) for o in outs]).astype(np.float32)
    return out


# revision 28
# speedup vs baseline: 1.1332x; 1.1332x over previous
"""Trainium2 Bass kernel for nn_Block_26628797235524 (Mamba-style cross-scan SSM block).

Sharding: batch B=8 -> one batch element per NeuronCore (8 cores, SPMD, no
collectives). Each core runs the full block for its batch element:
  in_proj -> conv(1x1x1)+silu -> dual-order selective scan (K=2, DIN=256,
  DST=16) -> combine -> layernorm -> gate -> out_proj.

v2: fp16 hot path (DVE 2-byte fast modes, 1-cyc/row PE matmuls), fused
Silu/Softplus activations, Ds skip-connection folded into the PSUM
accumulation via a diagonal matmul, xs/z kept resident in SBUF (no DRAM
round-trips), fp16 B/C broadcasts. Scan runs as tensor_tensor_scan over
128-channel x 1024-step slabs chained via the last column; state is fp32
inside the scan instruction so fp16 operands only quantize the readout.

kernel(**inputs) takes the FULL unsharded inputs and returns the FULL output.
"""

import os
import sys
from contextlib import ExitStack

import numpy as np

_RL = "/opt/trn_rl_repo"
if os.path.isdir(_RL) and _RL not in sys.path:
    sys.path.insert(0, _RL)

import concourse.bass as bass
import concourse.bacc as bacc
import concourse.tile as tile
from concourse import mybir
from concourse.bass_utils import run_bass_kernel_spmd

# Problem sizes (hardcoded per the task contract).
B, T, H, W, DIM = 8, 16, 16, 16, 128
DIN, DST, DTR, KG = 256, 16, 8, 2
L = T * H * W          # 4096
P = 128                # partitions
DH = DIN // P          # 2 d-half tiles per direction
LC = 1024              # scan slab length
NQ = L // LC           # 4
NCORES = 8

F32 = mybir.dt.float32
F16 = mybir.dt.float16
AF = mybir.ActivationFunctionType
ALU = mybir.AluOpType
MM_F = 512             # matmul free-dim chunk (one PSUM bank)
NMM = L // MM_F        # 8 chunks over L

# Engine split knobs for the scan inner loop (tuned from traces).
# Scans are DVE-only (backend rejects Pool TensorScalarPtr). GpSimd shares
# SBUF ports with DVE, so concurrent gpsimd work degrades DVE throughput —
# offload only a moderate slice of the C-side multiplies.
TMP_ON_GPSIMD = lambda n, i: (n % 2) == 0
XIN_ON_GPSIMD = lambda n, i: False


def _declare_drams(nc):
    d = {}

    def inp(name, shape, dt=F16):
        d[name] = nc.dram_tensor(name, list(shape), dt, kind="ExternalInput")

    inp("xT", (P, L))                       # per-core batch slice, channel-major
    inp("w_in", (P, 4 * P))                 # in_proj_w.T
    inp("conv_sc", (DH, P, 1), F32)
    inp("conv_bi", (DH, P, 1), F32)
    inp("w_xproj", (KG, DH, P, 40))         # x_proj_w[k].T in 2 pi-chunks
    inp("w_dt", (KG, DTR, DIN))             # dt_w[k].T
    inp("dt_bias", (KG, DH, P, 1), F32)
    inp("a_mat", (KG, DH, P, DST), F32)     # A = -exp(A_logs)
    inp("ds_diag", (KG, DH, P, P))          # diag(Ds) per (k, half)
    inp("lnw", (DH, P, 1), F32)
    inp("lnb", (DH, P, 1), F32)
    inp("w_out", (DH, P, P))                # out_proj_w.T in 2 pi-chunks
    inp("ident", (P, P))                    # identity: PE accumulate matmuls
    d["bc_dram"] = nc.dram_tensor("bc_dram", [KG, 2 * DST, L], F16)  # B/C rows
    d["outT"] = nc.dram_tensor("outT", [P, L], F32, kind="ExternalOutput")
    return d


def _body(tc, d):
    nc = tc.nc
    with ExitStack() as ctx:
        const = ctx.enter_context(tc.tile_pool(name="const", bufs=1))

        # ---- constants ----
        w_in = const.tile([P, 4 * P], F16, tag="w_in", name="w_in")
        nc.sync.dma_start(w_in[:], d["w_in"][:])
        conv_sc = [const.tile([P, 1], F32, tag=f"csc{i}", name=f"csc{i}") for i in range(DH)]
        conv_bi = [const.tile([P, 1], F32, tag=f"cbi{i}", name=f"cbi{i}") for i in range(DH)]
        for i in range(DH):
            nc.sync.dma_start(conv_sc[i][:], d["conv_sc"][i])
            nc.sync.dma_start(conv_bi[i][:], d["conv_bi"][i])
        w_xproj = [[const.tile([P, 40], F16, tag=f"wxp{k}{i}", name=f"wxp{k}{i}") for i in range(DH)]
                   for k in range(KG)]
        w_dt = [const.tile([DTR, DIN], F16, tag=f"wdt{k}", name=f"wdt{k}") for k in range(KG)]
        dt_bias = [[const.tile([P, 1], F32, tag=f"dtb{k}{i}", name=f"dtb{k}{i}") for i in range(DH)]
                   for k in range(KG)]
        a_mat = [[const.tile([P, DST], F32, tag=f"am{k}{i}", name=f"am{k}{i}") for i in range(DH)]
                 for k in range(KG)]
        ds_diag = [[const.tile([P, P], F16, tag=f"dsd{k}{i}", name=f"dsd{k}{i}") for i in range(DH)]
                   for k in range(KG)]
        for k in range(KG):
            nc.sync.dma_start(w_dt[k][:], d["w_dt"][k])
            for i in range(DH):
                nc.sync.dma_start(w_xproj[k][i][:], d["w_xproj"][k, i])
                nc.sync.dma_start(dt_bias[k][i][:], d["dt_bias"][k, i])
                nc.sync.dma_start(a_mat[k][i][:], d["a_mat"][k, i])
                nc.sync.dma_start(ds_diag[k][i][:], d["ds_diag"][k, i])
        lnw = [const.tile([P, 1], F32, tag=f"lnw{i}", name=f"lnw{i}") for i in range(DH)]
        lnb = [const.tile([P, 1], F32, tag=f"lnb{i}", name=f"lnb{i}") for i in range(DH)]
        w_out = [const.tile([P, P], F16, tag=f"wo{i}", name=f"wo{i}") for i in range(DH)]
        for i in range(DH):
            nc.sync.dma_start(lnw[i][:], d["lnw"][i])
            nc.sync.dma_start(lnb[i][:], d["lnb"][i])
            nc.sync.dma_start(w_out[i][:], d["w_out"][i])
        # 1/DIN-scaled ones column: the LN mean/second-moment contraction.
        oneN_col = const.tile([P, 1], F16, tag="oneN_col", name="oneN_col")
        nc.vector.memset(oneN_col[:], 1.0 / DIN)
        ones_row = const.tile([1, P], F16, tag="ones_row", name="ones_row")
        nc.vector.memset(ones_row[:], 1.0)
        ident = const.tile([P, P], F16, tag="ident", name="ident")
        nc.sync.dma_start(ident[:], d["ident"][:])
        eps = const.tile([1, 1], F32, tag="eps", name="eps")
        nc.vector.memset(eps[:], 1e-5)

        # ---- persistent activations (fp16 [P, L] = 8KB/partition each) ----
        main = ctx.enter_context(tc.tile_pool(name="main", bufs=1))
        xs = [[main.tile([P, L], F16, tag=f"xs{k}{i}", name=f"xs{k}{i}")
               for i in range(DH)] for k in range(KG)]
        z_sb = [main.tile([P, L], F16, tag=f"z{i}", name=f"z{i}") for i in range(DH)]
        y0_sb = [main.tile([P, L], F16, tag=f"y0_{i}", name=f"y0_{i}")
                 for i in range(DH)]
        ysum = [main.tile([P, L], F16, tag=f"ys{i}", name=f"ys{i}")
                for i in range(DH)]

        # ========== Phase 1: in_proj -> conv+silu -> scan orderings ==========
        with tc.tile_pool(name="p1", bufs=1) as p1pool, \
             tc.tile_pool(name="p1ps", bufs=4, space=bass.MemorySpace.PSUM) as p1ps:
            xT = p1pool.tile([P, L], F16, tag="xT", name="xT")
            nc.sync.dma_start(xT[:], d["xT"][:])
            for po in range(4):
                for c in range(NMM):
                    cs = slice(c * MM_F, (c + 1) * MM_F)
                    pt = p1ps.tile([P, MM_F], F32, tag="mm", name="mm")
                    nc.tensor.matmul(pt[:], w_in[:, po * P:(po + 1) * P],
                                     xT[:, cs], start=True, stop=True)
                    if po < DH:
                        # xh = silu(xz*conv_w + conv_b), written twice:
                        # (h w t) order for k=0 and reversed (t h w) for k=1.
                        src3 = pt[:].rearrange("p (t hw) -> p t hw", t=2, hw=H * W)
                        dst3 = xs[0][po][:].rearrange(
                            "p (hw t) -> p t hw", hw=H * W, t=T)[:, 2 * c:2 * c + 2, :]
                        nc.scalar.activation(dst3, src3, AF.Silu,
                                             bias=conv_bi[po][:], scale=conv_sc[po][:])
                        rev = xs[1][po][:, ::-1]
                        nc.scalar.activation(rev[:, cs], pt[:], AF.Silu,
                                             bias=conv_bi[po][:], scale=conv_sc[po][:])
                    else:
                        nc.scalar.activation(z_sb[po - DH][:, cs], pt[:], AF.Silu)

        # ================= Per-direction pipeline =================
        for k in range(KG):
            with tc.tile_pool(name=f"kp{k}", bufs=1) as kpool:
                delta = [kpool.tile([P, L], F16, tag=f"delta{i}", name=f"delta{i}")
                         for i in range(DH)]
                dU = [kpool.tile([P, L], F16, tag=f"dU{i}", name=f"dU{i}")
                      for i in range(DH)]
                # ---- Phase 3: x_dbl -> (dts, B, C); delta = softplus ----
                with tc.tile_pool(name=f"kd{k}", bufs=1) as kdpool, \
                     tc.tile_pool(name=f"kps{k}", bufs=1,
                                  space=bass.MemorySpace.PSUM) as kps:
                    xdbl = kdpool.tile([DTR, L], F16, tag="xdbl", name="xdbl")
                    for c in range(NMM):
                        cs = slice(c * MM_F, (c + 1) * MM_F)
                        ptA = kps.tile([DTR, MM_F], F32, tag="mmA", name="mmA")
                        nc.tensor.matmul(ptA[:], w_xproj[k][0][:, 0:DTR],
                                         xs[k][0][:, cs], start=True, stop=False)
                        nc.tensor.matmul(ptA[:], w_xproj[k][1][:, 0:DTR],
                                         xs[k][1][:, cs], start=False, stop=True)
                        nc.scalar.activation(xdbl[:, cs], ptA[:], AF.Copy)
                        ptB = kps.tile([2 * DST, MM_F], F32, tag="mmB", name="mmB")
                        nc.tensor.matmul(ptB[:], w_xproj[k][0][:, DTR:40],
                                         xs[k][0][:, cs], start=True, stop=False)
                        nc.tensor.matmul(ptB[:], w_xproj[k][1][:, DTR:40],
                                         xs[k][1][:, cs], start=False, stop=True)
                        bcs = kdpool.tile([2 * DST, MM_F], F16, tag="bcs",
                                          name="bcs", bufs=3)
                        nc.scalar.activation(bcs[:], ptB[:], AF.Copy)
                        nc.sync.dma_start(d["bc_dram"][k, :, cs], bcs[:])
                    for i in range(DH):
                        for c in range(NMM):
                            cs = slice(c * MM_F, (c + 1) * MM_F)
                            pt2 = kps.tile([P, MM_F], F32, tag="mmd", name="mmd")
                            nc.tensor.matmul(pt2[:], w_dt[k][:, i * P:(i + 1) * P],
                                             xdbl[:, cs], start=True, stop=True)
                            # softplus(x + b) = ln(1 + exp(x + b))
                            ed = kdpool.tile([P, MM_F], F16, tag="ed",
                                             name="ed", bufs=3)
                            nc.scalar.activation(ed[:], pt2[:], AF.Exp,
                                                 bias=dt_bias[k][i][:])
                            nc.scalar.activation(delta[i][:, cs], ed[:],
                                                 AF.Ln, bias=1.0)
                    for i in range(DH):
                        nc.vector.tensor_tensor(dU[i][:], delta[i][:], xs[k][i][:],
                                                ALU.mult)

                # ---- Phase 4: selective scan (q -> n -> i) ----
                states = [kpool.tile([P, DST], F16, tag=f"st{i}", name=f"st{i}")
                          for i in range(DH)]
                with tc.tile_pool(name=f"sc{k}", bufs=2) as work, \
                     tc.tile_pool(name=f"bc{k}", bufs=3) as bcp, \
                     tc.tile_pool(name=f"scps{k}", bufs=2,
                                  space=bass.MemorySpace.PSUM) as scps:
                    for q in range(NQ):
                        sl = slice(q * LC, (q + 1) * LC)
                        y_ps = [scps.tile([P, LC], F32, tag=f"yps{i}",
                                          name=f"yps{i}") for i in range(DH)]
                        for i in range(DH):
                            for hb in range(LC // MM_F):
                                ps_ = slice(hb * MM_F, (hb + 1) * MM_F)
                                gs = slice(q * LC + hb * MM_F,
                                           q * LC + (hb + 1) * MM_F)
                                nc.tensor.matmul(y_ps[i][:, ps_], ds_diag[k][i][:],
                                                 xs[k][i][:, gs],
                                                 start=True, stop=False)
                        for n in range(DST):
                            brep = bcp.tile([P, LC], F16, tag="brep", name="brep")
                            nc.sync.dma_start(
                                brep[:],
                                d["bc_dram"][k, n:n + 1, sl].partition_broadcast(P))
                            crep = bcp.tile([P, LC], F16, tag="crep", name="crep")
                            nc.sync.dma_start(
                                crep[:],
                                d["bc_dram"][k, DST + n:DST + n + 1,
                                             sl].partition_broadcast(P))
                            for i in range(DH):
                                dA = work.tile([P, LC], F16, tag=f"dA{i}",
                                               name=f"dA{i}")
                                nc.scalar.activation(dA[:], delta[i][:, sl], AF.Exp,
                                                     scale=a_mat[k][i][:, n:n + 1])
                                xin = work.tile([P, LC], F16, tag="xin", name="xin")
                                xeng = nc.gpsimd if XIN_ON_GPSIMD(n, i) else nc.vector
                                xeng.tensor_tensor(xin[:], dU[i][:, sl],
                                                   brep[:], ALU.mult)
                                h = work.tile([P, LC], F16, tag=f"h{i}",
                                              name=f"h{i}")
                                init = 0.0 if q == 0 else states[i][:, n:n + 1]
                                nc.vector.tensor_tensor_scan(
                                    h[:], dA[:], xin[:], init, ALU.mult, ALU.add)
                                if q < NQ - 1:
                                    nc.vector.tensor_copy(states[i][:, n:n + 1],
                                                          h[:, LC - 1:LC])
                                tmp = work.tile([P, LC], F16, tag="tmp", name="tmp")
                                eng = nc.gpsimd if TMP_ON_GPSIMD(n, i) else nc.vector
                                eng.tensor_tensor(tmp[:], crep[:], h[:], ALU.mult)
                                for hb in range(LC // MM_F):
                                    ps_ = slice(hb * MM_F, (hb + 1) * MM_F)
                                    nc.tensor.matmul(y_ps[i][:, ps_], ident[:],
                                                     tmp[:, ps_],
                                                     start=False,
                                                     stop=(n == DST - 1))
                        for i in range(DH):
                            if k == 0:
                                nc.scalar.activation(y0_sb[i][:, sl], y_ps[i][:],
                                                     AF.Copy)
                            else:
                                # Fused drain+combine: ysum natural slice =
                                # reordered y_fwd + reversed y_rvs (PSUM read).
                                tq = (NQ - 1 - q) * (LC // (H * W))
                                nsl = slice((NQ - 1 - q) * LC, (NQ - q) * LC)
                                src0 = y0_sb[i][:].rearrange(
                                    "p (hw t) -> p t hw", hw=H * W,
                                    t=T)[:, tq:tq + LC // (H * W), :]
                                src1 = y_ps[i][:, ::-1].rearrange(
                                    "p (t hw) -> p t hw", t=LC // (H * W),
                                    hw=H * W)
                                dst = ysum[i][:, nsl].rearrange(
                                    "p (t hw) -> p t hw", t=LC // (H * W),
                                    hw=H * W)
                                nc.vector.tensor_tensor(dst, src0, src1,
                                                        ALU.add)

        # ================= Phase 5-7: LN, gate, out_proj =================
        with tc.tile_pool(name="fin", bufs=1) as fin:
            # LN stats over DIN (partition reduce via PE 1/DIN-ones contraction)
            mu16 = fin.tile([1, L], F16, tag="mu16", name="mu16")
            m2_16 = fin.tile([1, L], F16, tag="m2_16", name="m2_16")
            sq16 = fin.tile([1, L], F16, tag="sq16", name="sq16")
            rs16 = fin.tile([1, L], F16, tag="rs16", name="rs16")
            with tc.tile_pool(name="fps1", bufs=4,
                              space=bass.MemorySpace.PSUM) as fps1:
                for c in range(NMM):
                    cs = slice(c * MM_F, (c + 1) * MM_F)
                    pmu = fps1.tile([1, MM_F], F32, tag="pmu", name="pmu")
                    nc.tensor.matmul(pmu[:], oneN_col[:], ysum[0][:, cs],
                                     start=True, stop=False)
                    nc.tensor.matmul(pmu[:], oneN_col[:], ysum[1][:, cs],
                                     start=False, stop=True)
                    nc.scalar.activation(mu16[:, cs], pmu[:], AF.Copy)
                    psq = fps1.tile([1, MM_F], F32, tag="psq", name="psq")
                    for i in range(DH):
                        ysq = fin.tile([P, MM_F], F16, tag="ysq", name="ysq",
                                       bufs=2)
                        nc.scalar.activation(ysq[:], ysum[i][:, cs], AF.Square)
                        nc.tensor.matmul(psq[:], oneN_col[:], ysq[:],
                                         start=(i == 0), stop=(i == DH - 1))
                    nc.scalar.activation(m2_16[:, cs], psq[:], AF.Copy)
            # var = E[y^2] - mu^2 ; rstd = exp(-0.5*ln(var + eps))
            nc.vector.tensor_tensor(sq16[:], mu16[:], mu16[:], ALU.mult)
            nc.vector.tensor_tensor(m2_16[:], m2_16[:], sq16[:], ALU.subtract)
            nc.scalar.activation(sq16[:], m2_16[:], AF.Ln, bias=eps[:1, :])
            nc.scalar.activation(rs16[:], sq16[:], AF.Exp, scale=-0.5)

            # normalize + affine + gate + out_proj, chunked over L
            with tc.tile_pool(name="fch", bufs=2) as fch, \
                 tc.tile_pool(name="fps2", bufs=2,
                              space=bass.MemorySpace.PSUM) as fps2:
                for c in range(NMM):
                    cs = slice(c * MM_F, (c + 1) * MM_F)
                    pm = fps2.tile([P, MM_F], F32, tag="pm", name="pm")
                    nc.tensor.matmul(pm[:], ones_row[:], mu16[:, cs],
                                     start=True, stop=True)
                    mrep = fch.tile([P, MM_F], F16, tag="mrep", name="mrep")
                    nc.scalar.activation(mrep[:], pm[:], AF.Copy)
                    pr = fps2.tile([P, MM_F], F32, tag="pr", name="pr")
                    nc.tensor.matmul(pr[:], ones_row[:], rs16[:, cs],
                                     start=True, stop=True)
                    rrep = fch.tile([P, MM_F], F16, tag="rrep", name="rrep")
                    nc.scalar.activation(rrep[:], pr[:], AF.Copy)
                    g = []
                    for i in range(DH):
                        yc = fch.tile([P, MM_F], F16, tag="yc", name="yc")
                        nc.vector.tensor_tensor(yc[:], ysum[i][:, cs], mrep[:],
                                                ALU.subtract)
                        yn = fch.tile([P, MM_F], F16, tag="yn", name="yn")
                        nc.vector.tensor_tensor(yn[:], yc[:], rrep[:], ALU.mult)
                        ya = fch.tile([P, MM_F], F16, tag="ya", name="ya")
                        nc.scalar.activation(ya[:], yn[:], AF.Identity,
                                             bias=lnb[i][:], scale=lnw[i][:])
                        gi = fch.tile([P, MM_F], F16, tag=f"g{i}", name=f"g{i}")
                        nc.vector.tensor_tensor(gi[:], ya[:], z_sb[i][:, cs],
                                                ALU.mult)
                        g.append(gi)
                    po = fps2.tile([P, MM_F], F32, tag="pout", name="pout")
                    nc.tensor.matmul(po[:], w_out[0][:], g[0][:],
                                     start=True, stop=False)
                    nc.tensor.matmul(po[:], w_out[1][:], g[1][:],
                                     start=False, stop=True)
                    osb = fch.tile([P, MM_F], F32, tag="osb", name="osb")
                    nc.scalar.activation(osb[:], po[:], AF.Copy)
                    nc.sync.dma_start(d["outT"][:, cs], osb[:])


_CACHE = {}


def _get_program():
    if "nc" not in _CACHE:
        nc = bacc.Bacc("TRN2", target_bir_lowering=False, debug=False,
                       num_devices=NCORES)
        d = _declare_drams(nc)
        with tile.TileContext(nc) as tc:
            _body(tc, d)
        nc.compile()
        _CACHE["nc"] = nc
    return _CACHE["nc"]


def _host_weights(inputs):
    f32 = lambda a: np.ascontiguousarray(np.asarray(a, np.float32))
    f16 = lambda a: np.ascontiguousarray(np.asarray(a, np.float32).astype(np.float16))
    in_proj_w = f32(inputs["in_proj_w"])        # (512, 128)
    x_proj_w = f32(inputs["x_proj_w"])          # (2, 40, 256)
    dt_w = f32(inputs["dt_w"])                  # (2, 256, 8)
    dt_b = f32(inputs["dt_b"])                  # (2, 256)
    A_logs = f32(inputs["A_logs"])              # (512, 16)
    Ds = f32(inputs["Ds"])                      # (512,)
    ds_diag = np.zeros((KG, DH, P, P), np.float16)
    dsr = Ds.reshape(KG, DH, P)
    for k in range(KG):
        for i in range(DH):
            np.fill_diagonal(ds_diag[k, i], dsr[k, i].astype(np.float16))
    m = {
        "w_in": f16(in_proj_w.T),                                   # (128, 512)
        "conv_sc": f32(inputs["conv_w"]).reshape(DH, P, 1),
        "conv_bi": f32(inputs["conv_b"]).reshape(DH, P, 1),
        "w_xproj": f16(x_proj_w.transpose(0, 2, 1).reshape(KG, DH, P, 40)),
        "w_dt": f16(dt_w.transpose(0, 2, 1)),                       # (2, 8, 256)
        "dt_bias": f32(dt_b).reshape(KG, DH, P, 1),
        "a_mat": f32(-np.exp(A_logs)).reshape(KG, DH, P, DST),
        "ds_diag": ds_diag,
        "lnw": f32(inputs["ln_w"]).reshape(DH, P, 1),
        "lnb": f32(inputs["ln_b"]).reshape(DH, P, 1),
        "w_out": f16(f32(inputs["out_proj_w"]).T.reshape(DH, P, P)),
        "ident": np.eye(P, dtype=np.float16),
    }
    return m


def kernel(**inputs):
    x = np.ascontiguousarray(np.asarray(inputs["x"], np.float32))   # (8,16,16,16,128)
    shared = _host_weights(inputs)
    nc = _get_program()
    in_maps = []
    for b in range(NCORES):
        m = dict(shared)
        m["xT"] = np.ascontiguousarray(x[b].reshape(L, DIM).T).astype(np.float16)
        in_maps.append(m)
    trace = bool(int(os.environ.get("BASS_PROFILE", "0")))
    res = run_bass_kernel_spmd(nc, in_maps, list(range(NCORES)), trace=trace)
    _CACHE["last_result"] = res
    outs = [r["outT"] for r in res.results]
    out = np.stack([o.T.reshape(T, H, W, DIM) for o in outs]).astype(np.float32)
    return out


# revision 31
# speedup vs baseline: 1.2123x; 1.0698x over previous
"""Trainium2 Bass kernel for nn_Block_26628797235524 (Mamba-style cross-scan SSM block).

Sharding: batch B=8 -> one batch element per NeuronCore (8 cores, SPMD, no
collectives). Each core runs the full block for its batch element:
  in_proj -> conv(1x1x1)+silu -> dual-order selective scan (K=2, DIN=256,
  DST=16) -> combine -> layernorm -> gate -> out_proj.

v2: fp16 hot path (DVE 2-byte fast modes, 1-cyc/row PE matmuls), fused
Silu/Softplus activations, Ds skip-connection folded into the PSUM
accumulation via a diagonal matmul, xs/z kept resident in SBUF (no DRAM
round-trips), fp16 B/C broadcasts. Scan runs as tensor_tensor_scan over
128-channel x 1024-step slabs chained via the last column; state is fp32
inside the scan instruction so fp16 operands only quantize the readout.

kernel(**inputs) takes the FULL unsharded inputs and returns the FULL output.
"""

import os
import sys
from contextlib import ExitStack

import numpy as np

_RL = "/opt/trn_rl_repo"
if os.path.isdir(_RL) and _RL not in sys.path:
    sys.path.insert(0, _RL)

import concourse.bass as bass
import concourse.bacc as bacc
import concourse.tile as tile
from concourse import mybir
from concourse.bass_utils import run_bass_kernel_spmd

# Problem sizes (hardcoded per the task contract).
B, T, H, W, DIM = 8, 16, 16, 16, 128
DIN, DST, DTR, KG = 256, 16, 8, 2
L = T * H * W          # 4096
P = 128                # partitions
DH = DIN // P          # 2 d-half tiles per direction
LC = 1024              # scan slab length
NQ = L // LC           # 4
NCORES = 8

F32 = mybir.dt.float32
F16 = mybir.dt.float16
AF = mybir.ActivationFunctionType
ALU = mybir.AluOpType
MM_F = 512             # matmul free-dim chunk (one PSUM bank)
NMM = L // MM_F        # 8 chunks over L

# Engine split knobs for the scan inner loop (tuned from traces).
# Scans are DVE-only (backend rejects Pool TensorScalarPtr). GpSimd shares
# SBUF ports with DVE, so concurrent gpsimd work degrades DVE throughput —
# offload only a moderate slice of the C-side multiplies.
TMP_ON_GPSIMD = lambda n, i: (n % 2) == 0
XIN_ON_GPSIMD = lambda n, i: False


def _declare_drams(nc):
    d = {}

    def inp(name, shape, dt=F16):
        d[name] = nc.dram_tensor(name, list(shape), dt, kind="ExternalInput")

    inp("xT", (P, L))                       # per-core batch slice, channel-major
    inp("w_in", (P, 4 * P))                 # in_proj_w.T
    inp("conv_sc", (DH, P, 1), F32)
    inp("conv_bi", (DH, P, 1), F32)
    inp("w_xproj", (KG, DH, P, 40))         # x_proj_w[k].T in 2 pi-chunks
    inp("w_dt", (KG, DTR, DIN))             # dt_w[k].T
    inp("dt_bias", (KG, DH, P, 1), F32)
    inp("a_mat", (KG, DH, P, DST), F32)     # A = -exp(A_logs)
    inp("ds_diag", (KG, DH, P, P))          # diag(Ds) per (k, half)
    inp("lnw", (DH, P, 1), F32)
    inp("lnb", (DH, P, 1), F32)
    inp("w_out", (DH, P, P))                # out_proj_w.T in 2 pi-chunks
    inp("ident", (P, P))                    # identity: PE accumulate matmuls
    d["bc_dram"] = nc.dram_tensor("bc_dram", [KG, 2 * DST, L], F16)  # B/C rows
    d["outT"] = nc.dram_tensor("outT", [P, L], F32, kind="ExternalOutput")
    return d


def _body(tc, d):
    nc = tc.nc
    with ExitStack() as ctx:
        const = ctx.enter_context(tc.tile_pool(name="const", bufs=1))

        # ---- constants ----
        w_in = const.tile([P, 4 * P], F16, tag="w_in", name="w_in")
        nc.sync.dma_start(w_in[:], d["w_in"][:])
        conv_sc = [const.tile([P, 1], F32, tag=f"csc{i}", name=f"csc{i}") for i in range(DH)]
        conv_bi = [const.tile([P, 1], F32, tag=f"cbi{i}", name=f"cbi{i}") for i in range(DH)]
        for i in range(DH):
            nc.sync.dma_start(conv_sc[i][:], d["conv_sc"][i])
            nc.sync.dma_start(conv_bi[i][:], d["conv_bi"][i])
        w_xproj = [[const.tile([P, 40], F16, tag=f"wxp{k}{i}", name=f"wxp{k}{i}") for i in range(DH)]
                   for k in range(KG)]
        w_dt = [const.tile([DTR, DIN], F16, tag=f"wdt{k}", name=f"wdt{k}") for k in range(KG)]
        dt_bias = [[const.tile([P, 1], F32, tag=f"dtb{k}{i}", name=f"dtb{k}{i}") for i in range(DH)]
                   for k in range(KG)]
        a_mat = [[const.tile([P, DST], F32, tag=f"am{k}{i}", name=f"am{k}{i}") for i in range(DH)]
                 for k in range(KG)]
        ds_diag = [[const.tile([P, P], F16, tag=f"dsd{k}{i}", name=f"dsd{k}{i}") for i in range(DH)]
                   for k in range(KG)]
        for k in range(KG):
            nc.sync.dma_start(w_dt[k][:], d["w_dt"][k])
            for i in range(DH):
                nc.sync.dma_start(w_xproj[k][i][:], d["w_xproj"][k, i])
                nc.sync.dma_start(dt_bias[k][i][:], d["dt_bias"][k, i])
                nc.sync.dma_start(a_mat[k][i][:], d["a_mat"][k, i])
                nc.sync.dma_start(ds_diag[k][i][:], d["ds_diag"][k, i])
        lnw = [const.tile([P, 1], F32, tag=f"lnw{i}", name=f"lnw{i}") for i in range(DH)]
        lnb = [const.tile([P, 1], F32, tag=f"lnb{i}", name=f"lnb{i}") for i in range(DH)]
        w_out = [const.tile([P, P], F16, tag=f"wo{i}", name=f"wo{i}") for i in range(DH)]
        for i in range(DH):
            nc.sync.dma_start(lnw[i][:], d["lnw"][i])
            nc.sync.dma_start(lnb[i][:], d["lnb"][i])
            nc.sync.dma_start(w_out[i][:], d["w_out"][i])
        # 1/DIN-scaled ones column: the LN mean/second-moment contraction.
        oneN_col = const.tile([P, 1], F16, tag="oneN_col", name="oneN_col")
        nc.vector.memset(oneN_col[:], 1.0 / DIN)
        ones_row = const.tile([1, P], F16, tag="ones_row", name="ones_row")
        nc.vector.memset(ones_row[:], 1.0)
        ident = const.tile([P, P], F16, tag="ident", name="ident")
        nc.sync.dma_start(ident[:], d["ident"][:])
        eps = const.tile([1, 1], F32, tag="eps", name="eps")
        nc.vector.memset(eps[:], 1e-5)

        # ---- persistent activations (fp16 [P, L] = 8KB/partition each) ----
        main = ctx.enter_context(tc.tile_pool(name="main", bufs=1))
        xs = [[main.tile([P, L], F16, tag=f"xs{k}{i}", name=f"xs{k}{i}")
               for i in range(DH)] for k in range(KG)]
        z_sb = [main.tile([P, L], F16, tag=f"z{i}", name=f"z{i}") for i in range(DH)]
        y0_sb = [main.tile([P, L], F16, tag=f"y0_{i}", name=f"y0_{i}")
                 for i in range(DH)]
        ysum = [main.tile([P, L], F16, tag=f"ys{i}", name=f"ys{i}")
                for i in range(DH)]

        # ========== Phase 1: in_proj -> conv+silu -> scan orderings ==========
        with tc.tile_pool(name="p1", bufs=1) as p1pool, \
             tc.tile_pool(name="p1ps", bufs=4, space=bass.MemorySpace.PSUM) as p1ps:
            xT = p1pool.tile([P, L], F16, tag="xT", name="xT")
            nc.sync.dma_start(xT[:], d["xT"][:])
            for po in range(4):
                for c in range(NMM):
                    cs = slice(c * MM_F, (c + 1) * MM_F)
                    pt = p1ps.tile([P, MM_F], F32, tag="mm", name="mm")
                    nc.tensor.matmul(pt[:], w_in[:, po * P:(po + 1) * P],
                                     xT[:, cs], start=True, stop=True)
                    if po < DH:
                        # xh = silu(xz*conv_w + conv_b), written twice:
                        # (h w t) order for k=0 and reversed (t h w) for k=1.
                        src3 = pt[:].rearrange("p (t hw) -> p t hw", t=2, hw=H * W)
                        dst3 = xs[0][po][:].rearrange(
                            "p (hw t) -> p t hw", hw=H * W, t=T)[:, 2 * c:2 * c + 2, :]
                        nc.scalar.activation(dst3, src3, AF.Silu,
                                             bias=conv_bi[po][:], scale=conv_sc[po][:])
                        rev = xs[1][po][:, ::-1]
                        nc.scalar.activation(rev[:, cs], pt[:], AF.Silu,
                                             bias=conv_bi[po][:], scale=conv_sc[po][:])
                    else:
                        nc.scalar.activation(z_sb[po - DH][:, cs], pt[:], AF.Silu)

        # ================= Per-direction pipeline =================
        delta = [[main.tile([P, L], F16, tag=f"delta{k}{i}", name=f"delta{k}{i}")
                  for i in range(DH)] for k in range(KG)]
        dU = [[main.tile([P, L], F16, tag=f"dU{k}{i}", name=f"dU{k}{i}")
               for i in range(DH)] for k in range(KG)]
        states = [main.tile([P, DST], F16, tag=f"st{i}", name=f"st{i}")
                  for i in range(DH)]

        # ---- Phase 3 (both directions): x_dbl -> (dts, B, C); softplus ----
        for k in range(KG):
            with tc.tile_pool(name=f"kd{k}", bufs=1) as kdpool, \
                 tc.tile_pool(name=f"kps{k}", bufs=2,
                              space=bass.MemorySpace.PSUM) as kps:
                xdbl = kdpool.tile([DTR, L], F16, tag="xdbl", name="xdbl")
                for c in range(NMM):
                    cs = slice(c * MM_F, (c + 1) * MM_F)
                    ptA = kps.tile([DTR, MM_F], F32, tag="mmA", name="mmA")
                    nc.tensor.matmul(ptA[:], w_xproj[k][0][:, 0:DTR],
                                     xs[k][0][:, cs], start=True, stop=False)
                    nc.tensor.matmul(ptA[:], w_xproj[k][1][:, 0:DTR],
                                     xs[k][1][:, cs], start=False, stop=True)
                    nc.scalar.activation(xdbl[:, cs], ptA[:], AF.Copy)
                    ptB = kps.tile([2 * DST, MM_F], F32, tag="mmB", name="mmB")
                    nc.tensor.matmul(ptB[:], w_xproj[k][0][:, DTR:40],
                                     xs[k][0][:, cs], start=True, stop=False)
                    nc.tensor.matmul(ptB[:], w_xproj[k][1][:, DTR:40],
                                     xs[k][1][:, cs], start=False, stop=True)
                    bcs = kdpool.tile([2 * DST, MM_F], F16, tag="bcs",
                                      name="bcs", bufs=3)
                    nc.scalar.activation(bcs[:], ptB[:], AF.Copy)
                    nc.sync.dma_start(d["bc_dram"][k, :, cs], bcs[:])
                # softplus(x + b) = ln(1 + exp(x + b)); batched so exp and ln
                # each load their activation table once, not per chunk.
                eds = [kdpool.tile([P, L], F16, tag=f"ed{i}", name=f"ed{i}")
                       for i in range(DH)]
                for i in range(DH):
                    for c in range(NMM):
                        cs = slice(c * MM_F, (c + 1) * MM_F)
                        pt2 = kps.tile([P, MM_F], F32, tag="mmd", name="mmd")
                        nc.tensor.matmul(pt2[:], w_dt[k][:, i * P:(i + 1) * P],
                                         xdbl[:, cs], start=True, stop=True)
                        nc.scalar.activation(eds[i][:, cs], pt2[:], AF.Exp,
                                             bias=dt_bias[k][i][:])
                for i in range(DH):
                    nc.scalar.activation(delta[k][i][:], eds[i][:], AF.Ln,
                                         bias=1.0)
                    nc.vector.tensor_tensor(dU[k][i][:], delta[k][i][:],
                                            xs[k][i][:], ALU.mult)

        # ---- Phase 4 (both directions): selective scan (q -> n -> i) ----
        with tc.tile_pool(name="sc", bufs=2) as work, \
             tc.tile_pool(name="bc", bufs=3) as bcp, \
             tc.tile_pool(name="scps", bufs=2,
                          space=bass.MemorySpace.PSUM) as scps:
            for k in range(KG):
                    for q in range(NQ):
                        sl = slice(q * LC, (q + 1) * LC)
                        y_ps = [scps.tile([P, LC], F32, tag=f"yps{i}",
                                          name=f"yps{i}") for i in range(DH)]
                        for i in range(DH):
                            for hb in range(LC // MM_F):
                                ps_ = slice(hb * MM_F, (hb + 1) * MM_F)
                                gs = slice(q * LC + hb * MM_F,
                                           q * LC + (hb + 1) * MM_F)
                                nc.tensor.matmul(y_ps[i][:, ps_], ds_diag[k][i][:],
                                                 xs[k][i][:, gs],
                                                 start=True, stop=False)
                        for n in range(DST):
                            brep = bcp.tile([P, LC], F16, tag="brep", name="brep")
                            nc.sync.dma_start(
                                brep[:],
                                d["bc_dram"][k, n:n + 1, sl].partition_broadcast(P))
                            crep = bcp.tile([P, LC], F16, tag="crep", name="crep")
                            nc.sync.dma_start(
                                crep[:],
                                d["bc_dram"][k, DST + n:DST + n + 1,
                                             sl].partition_broadcast(P))
                            for i in range(DH):
                                dA = work.tile([P, LC], F16, tag=f"dA{i}",
                                               name=f"dA{i}")
                                nc.scalar.activation(dA[:], delta[k][i][:, sl],
                                                     AF.Exp,
                                                     scale=a_mat[k][i][:, n:n + 1])
                                xin = work.tile([P, LC], F16, tag="xin", name="xin")
                                xeng = nc.gpsimd if XIN_ON_GPSIMD(n, i) else nc.vector
                                xeng.tensor_tensor(xin[:], dU[k][i][:, sl],
                                                   brep[:], ALU.mult)
                                h = work.tile([P, LC], F16, tag=f"h{i}",
                                              name=f"h{i}")
                                init = 0.0 if q == 0 else states[i][:, n:n + 1]
                                nc.vector.tensor_tensor_scan(
                                    h[:], dA[:], xin[:], init, ALU.mult, ALU.add)
                                if q < NQ - 1:
                                    nc.vector.tensor_copy(states[i][:, n:n + 1],
                                                          h[:, LC - 1:LC])
                                tmp = work.tile([P, LC], F16, tag="tmp", name="tmp")
                                eng = nc.gpsimd if TMP_ON_GPSIMD(n, i) else nc.vector
                                eng.tensor_tensor(tmp[:], crep[:], h[:], ALU.mult)
                                for hb in range(LC // MM_F):
                                    ps_ = slice(hb * MM_F, (hb + 1) * MM_F)
                                    nc.tensor.matmul(y_ps[i][:, ps_], ident[:],
                                                     tmp[:, ps_],
                                                     start=False,
                                                     stop=(n == DST - 1))
                        for i in range(DH):
                            if k == 0:
                                nc.scalar.activation(y0_sb[i][:, sl], y_ps[i][:],
                                                     AF.Copy)
                            else:
                                # Fused drain+combine: ysum natural slice =
                                # reordered y_fwd + reversed y_rvs (PSUM read).
                                tq = (NQ - 1 - q) * (LC // (H * W))
                                nsl = slice((NQ - 1 - q) * LC, (NQ - q) * LC)
                                src0 = y0_sb[i][:].rearrange(
                                    "p (hw t) -> p t hw", hw=H * W,
                                    t=T)[:, tq:tq + LC // (H * W), :]
                                src1 = y_ps[i][:, ::-1].rearrange(
                                    "p (t hw) -> p t hw", t=LC // (H * W),
                                    hw=H * W)
                                dst = ysum[i][:, nsl].rearrange(
                                    "p (t hw) -> p t hw", t=LC // (H * W),
                                    hw=H * W)
                                nc.vector.tensor_tensor(dst, src0, src1,
                                                        ALU.add)

        # ================= Phase 5-7: LN, gate, out_proj =================
        with tc.tile_pool(name="fin", bufs=1) as fin:
            # LN stats over DIN (partition reduce via PE 1/DIN-ones contraction)
            mu16 = fin.tile([1, L], F16, tag="mu16", name="mu16")
            m2_16 = fin.tile([1, L], F16, tag="m2_16", name="m2_16")
            sq16 = fin.tile([1, L], F16, tag="sq16", name="sq16")
            rs16 = fin.tile([1, L], F16, tag="rs16", name="rs16")
            with tc.tile_pool(name="fps1", bufs=4,
                              space=bass.MemorySpace.PSUM) as fps1:
                for c in range(NMM):
                    cs = slice(c * MM_F, (c + 1) * MM_F)
                    pmu = fps1.tile([1, MM_F], F32, tag="pmu", name="pmu")
                    nc.tensor.matmul(pmu[:], oneN_col[:], ysum[0][:, cs],
                                     start=True, stop=False)
                    nc.tensor.matmul(pmu[:], oneN_col[:], ysum[1][:, cs],
                                     start=False, stop=True)
                    nc.scalar.activation(mu16[:, cs], pmu[:], AF.Copy)
                    psq = fps1.tile([1, MM_F], F32, tag="psq", name="psq")
                    for i in range(DH):
                        ysq = fin.tile([P, MM_F], F16, tag="ysq", name="ysq",
                                       bufs=2)
                        nc.scalar.activation(ysq[:], ysum[i][:, cs], AF.Square)
                        nc.tensor.matmul(psq[:], oneN_col[:], ysq[:],
                                         start=(i == 0), stop=(i == DH - 1))
                    nc.scalar.activation(m2_16[:, cs], psq[:], AF.Copy)
            # var = E[y^2] - mu^2 ; rstd = exp(-0.5*ln(var + eps))
            nc.vector.tensor_tensor(sq16[:], mu16[:], mu16[:], ALU.mult)
            nc.vector.tensor_tensor(m2_16[:], m2_16[:], sq16[:], ALU.subtract)
            nc.scalar.activation(sq16[:], m2_16[:], AF.Ln, bias=eps[:1, :])
            nc.scalar.activation(rs16[:], sq16[:], AF.Exp, scale=-0.5)

            # normalize + affine + gate + out_proj, chunked over L
            with tc.tile_pool(name="fch", bufs=2) as fch, \
                 tc.tile_pool(name="fps2", bufs=2,
                              space=bass.MemorySpace.PSUM) as fps2:
                for c in range(NMM):
                    cs = slice(c * MM_F, (c + 1) * MM_F)
                    pm = fps2.tile([P, MM_F], F32, tag="pm", name="pm")
                    nc.tensor.matmul(pm[:], ones_row[:], mu16[:, cs],
                                     start=True, stop=True)
                    mrep = fch.tile([P, MM_F], F16, tag="mrep", name="mrep")
                    nc.scalar.activation(mrep[:], pm[:], AF.Copy)
                    pr = fps2.tile([P, MM_F], F32, tag="pr", name="pr")
                    nc.tensor.matmul(pr[:], ones_row[:], rs16[:, cs],
                                     start=True, stop=True)
                    rrep = fch.tile([P, MM_F], F16, tag="rrep", name="rrep")
                    nc.scalar.activation(rrep[:], pr[:], AF.Copy)
                    g = []
                    for i in range(DH):
                        yc = fch.tile([P, MM_F], F16, tag="yc", name="yc")
                        nc.vector.tensor_tensor(yc[:], ysum[i][:, cs], mrep[:],
                                                ALU.subtract)
                        yn = fch.tile([P, MM_F], F16, tag="yn", name="yn")
                        nc.vector.tensor_tensor(yn[:], yc[:], rrep[:], ALU.mult)
                        ya = fch.tile([P, MM_F], F16, tag="ya", name="ya")
                        nc.scalar.activation(ya[:], yn[:], AF.Identity,
                                             bias=lnb[i][:], scale=lnw[i][:])
                        gi = fch.tile([P, MM_F], F16, tag=f"g{i}", name=f"g{i}")
                        nc.vector.tensor_tensor(gi[:], ya[:], z_sb[i][:, cs],
                                                ALU.mult)
                        g.append(gi)
                    po = fps2.tile([P, MM_F], F32, tag="pout", name="pout")
                    nc.tensor.matmul(po[:], w_out[0][:], g[0][:],
                                     start=True, stop=False)
                    nc.tensor.matmul(po[:], w_out[1][:], g[1][:],
                                     start=False, stop=True)
                    osb = fch.tile([P, MM_F], F32, tag="osb", name="osb")
                    nc.scalar.activation(osb[:], po[:], AF.Copy)
                    nc.sync.dma_start(d["outT"][:, cs], osb[:])


_CACHE = {}


def _get_program():
    if "nc" not in _CACHE:
        nc = bacc.Bacc("TRN2", target_bir_lowering=False, debug=False,
                       num_devices=NCORES)
        d = _declare_drams(nc)
        with tile.TileContext(nc) as tc:
            _body(tc, d)
        nc.compile()
        _CACHE["nc"] = nc
    return _CACHE["nc"]


def _host_weights(inputs):
    f32 = lambda a: np.ascontiguousarray(np.asarray(a, np.float32))
    f16 = lambda a: np.ascontiguousarray(np.asarray(a, np.float32).astype(np.float16))
    in_proj_w = f32(inputs["in_proj_w"])        # (512, 128)
    x_proj_w = f32(inputs["x_proj_w"])          # (2, 40, 256)
    dt_w = f32(inputs["dt_w"])                  # (2, 256, 8)
    dt_b = f32(inputs["dt_b"])                  # (2, 256)
    A_logs = f32(inputs["A_logs"])              # (512, 16)
    Ds = f32(inputs["Ds"])                      # (512,)
    ds_diag = np.zeros((KG, DH, P, P), np.float16)
    dsr = Ds.reshape(KG, DH, P)
    for k in range(KG):
        for i in range(DH):
            np.fill_diagonal(ds_diag[k, i], dsr[k, i].astype(np.float16))
    m = {
        "w_in": f16(in_proj_w.T),                                   # (128, 512)
        "conv_sc": f32(inputs["conv_w"]).reshape(DH, P, 1),
        "conv_bi": f32(inputs["conv_b"]).reshape(DH, P, 1),
        "w_xproj": f16(x_proj_w.transpose(0, 2, 1).reshape(KG, DH, P, 40)),
        "w_dt": f16(dt_w.transpose(0, 2, 1)),                       # (2, 8, 256)
        "dt_bias": f32(dt_b).reshape(KG, DH, P, 1),
        "a_mat": f32(-np.exp(A_logs)).reshape(KG, DH, P, DST),
        "ds_diag": ds_diag,
        "lnw": f32(inputs["ln_w"]).reshape(DH, P, 1),
        "lnb": f32(inputs["ln_b"]).reshape(DH, P, 1),
        "w_out": f16(f32(inputs["out_proj_w"]).T.reshape(DH, P, P)),
        "ident": np.eye(P, dtype=np.float16),
    }
    return m


def kernel(**inputs):
    x = np.ascontiguousarray(np.asarray(inputs["x"], np.float32))   # (8,16,16,16,128)
    shared = _host_weights(inputs)
    nc = _get_program()
    in_maps = []
    for b in range(NCORES):
        m = dict(shared)
        m["xT"] = np.ascontiguousarray(x[b].reshape(L, DIM).T).astype(np.float16)
        in_maps.append(m)
    trace = bool(int(os.environ.get("BASS_PROFILE", "0")))
    res = run_bass_kernel_spmd(nc, in_maps, list(range(NCORES)), trace=trace)
    _CACHE["last_result"] = res
    outs = [r["outT"] for r in res.results]
    out = np.stack([o.T.reshape(T, H, W, DIM) for o in outs]).astype(np.float32)
    return out


# revision 49
# speedup vs baseline: 1.3029x; 1.0748x over previous
"""Trainium2 Bass kernel for nn_Block_26628797235524 (Mamba-style cross-scan SSM block).

Sharding: batch B=8 -> one batch element per NeuronCore (8 cores, SPMD, no
collectives). Each core runs the full block for its batch element:
  in_proj -> conv(1x1x1)+silu -> dual-order selective scan (K=2, DIN=256,
  DST=16) -> combine -> layernorm -> gate -> out_proj.

Optimizations over the naive pipeline (2.05ms -> ~1.24ms):
- fp16 hot path: DVE elementwise runs 2x vs fp32, PE matmuls 1 cyc/row vs 4.
  The scan (tensor_tensor_scan) keeps fp32 state internally, so fp16
  operands only quantize the readout (rel err ~8e-4 overall).
- Engine balance: scans are DVE-only (backend rejects Pool scans); half the
  C-side multiplies go to gpsimd. GpSimd shares SBUF ports with the DVE, so
  pushing more work there degrades DVE throughput (measured).
- Ds skip-connection folded into the PSUM accumulation as a diag(Ds) matmul;
  k=1 drain fused with the cross-scan combine (DVE reads PSUM directly).
- Everything SBUF-resident (no xs/z DRAM round-trips); B/C rows broadcast
  via fp16 partition-broadcast DMAs, double-buffered 4 deep.
- Activation-table discipline: exp/ln batched (softplus), copies kept on
  Copy (table-free); avoids the 1.3us table reload per exp<->ln switch.
- Software pipelining: phase 3 of direction k=1 and the z-gate silu are
  emitted as deferred units sprinkled into the k=0 scan loop; the LN/gate/
  out_proj epilogue is sprinkled per natural quarter into the k=1 scan loop
  (LN is per-position, so chunks finalize as their columns complete).

kernel(**inputs) takes the FULL unsharded inputs and returns the FULL output.
"""

import os
import sys
from contextlib import ExitStack

import numpy as np

_RL = "/opt/trn_rl_repo"
if os.path.isdir(_RL) and _RL not in sys.path:
    sys.path.insert(0, _RL)

import concourse.bass as bass
import concourse.bacc as bacc
import concourse.tile as tile
from concourse import mybir
from concourse.bass_utils import run_bass_kernel_spmd

# Problem sizes (hardcoded per the task contract).
B, T, H, W, DIM = 8, 16, 16, 16, 128
DIN, DST, DTR, KG = 256, 16, 8, 2
L = T * H * W          # 4096
P = 128                # partitions
DH = DIN // P          # 2 d-half tiles per direction
LC = 1024              # scan slab length
NQ = L // LC           # 4
NCORES = 8

F32 = mybir.dt.float32
F16 = mybir.dt.float16
AF = mybir.ActivationFunctionType
ALU = mybir.AluOpType
MM_F = 512             # matmul free-dim chunk (one PSUM bank)
NMM = L // MM_F        # 8 chunks over L

# Engine split knobs for the scan inner loop (tuned from traces).
# Scans are DVE-only (backend rejects Pool TensorScalarPtr). GpSimd shares
# SBUF ports with DVE, so concurrent gpsimd work degrades DVE throughput —
# offload only a moderate slice of the C-side multiplies.
TMP_ON_GPSIMD = lambda n, i: (n % 2) == 0
XIN_ON_GPSIMD = lambda n, i: False


def _declare_drams(nc):
    d = {}

    def inp(name, shape, dt=F16):
        d[name] = nc.dram_tensor(name, list(shape), dt, kind="ExternalInput")

    inp("xT", (P, L))                       # per-core batch slice, channel-major
    inp("w_in", (P, 4 * P))                 # in_proj_w.T
    inp("conv_sc", (DH, P, 1), F32)
    inp("conv_bi", (DH, P, 1), F32)
    inp("w_xproj", (KG, DH, P, 40))         # x_proj_w[k].T in 2 pi-chunks
    inp("w_dt", (KG, DTR, DIN))             # dt_w[k].T
    inp("dt_bias", (KG, DH, P, 1), F32)
    inp("a_mat", (KG, DH, P, DST), F32)     # A = -exp(A_logs)
    inp("ds_diag", (KG, DH, P, P))          # diag(Ds) per (k, half)
    inp("lnw", (DH, P, 1), F32)
    inp("lnb", (DH, P, 1), F32)
    inp("w_out", (DH, P, P))                # out_proj_w.T in 2 pi-chunks
    inp("ident", (P, P))                    # identity: PE accumulate matmuls
    d["bc_dram"] = nc.dram_tensor("bc_dram", [KG, 2 * DST, L], F16)  # B/C rows
    d["outT"] = nc.dram_tensor("outT", [P, L], F32, kind="ExternalOutput")
    return d


def _body(tc, d):
    nc = tc.nc
    with ExitStack() as ctx:
        const = ctx.enter_context(tc.tile_pool(name="const", bufs=1))

        # ---- constants ----
        w_in = const.tile([P, 4 * P], F16, tag="w_in", name="w_in")
        nc.sync.dma_start(w_in[:], d["w_in"][:])
        conv_sc = [const.tile([P, 1], F32, tag=f"csc{i}", name=f"csc{i}") for i in range(DH)]
        conv_bi = [const.tile([P, 1], F32, tag=f"cbi{i}", name=f"cbi{i}") for i in range(DH)]
        for i in range(DH):
            nc.sync.dma_start(conv_sc[i][:], d["conv_sc"][i])
            nc.sync.dma_start(conv_bi[i][:], d["conv_bi"][i])
        w_xproj = [[const.tile([P, 40], F16, tag=f"wxp{k}{i}", name=f"wxp{k}{i}") for i in range(DH)]
                   for k in range(KG)]
        w_dt = [const.tile([DTR, DIN], F16, tag=f"wdt{k}", name=f"wdt{k}") for k in range(KG)]
        dt_bias = [[const.tile([P, 1], F32, tag=f"dtb{k}{i}", name=f"dtb{k}{i}") for i in range(DH)]
                   for k in range(KG)]
        a_mat = [[const.tile([P, DST], F32, tag=f"am{k}{i}", name=f"am{k}{i}") for i in range(DH)]
                 for k in range(KG)]
        ds_diag = [[const.tile([P, P], F16, tag=f"dsd{k}{i}", name=f"dsd{k}{i}") for i in range(DH)]
                   for k in range(KG)]
        for k in range(KG):
            nc.sync.dma_start(w_dt[k][:], d["w_dt"][k])
            for i in range(DH):
                nc.sync.dma_start(w_xproj[k][i][:], d["w_xproj"][k, i])
                nc.sync.dma_start(dt_bias[k][i][:], d["dt_bias"][k, i])
                nc.sync.dma_start(a_mat[k][i][:], d["a_mat"][k, i])
                nc.sync.dma_start(ds_diag[k][i][:], d["ds_diag"][k, i])
        lnw = [const.tile([P, 1], F32, tag=f"lnw{i}", name=f"lnw{i}") for i in range(DH)]
        lnb = [const.tile([P, 1], F32, tag=f"lnb{i}", name=f"lnb{i}") for i in range(DH)]
        w_out = [const.tile([P, P], F16, tag=f"wo{i}", name=f"wo{i}") for i in range(DH)]
        for i in range(DH):
            nc.sync.dma_start(lnw[i][:], d["lnw"][i])
            nc.sync.dma_start(lnb[i][:], d["lnb"][i])
            nc.sync.dma_start(w_out[i][:], d["w_out"][i])
        # 1/DIN-scaled ones column: the LN mean/second-moment contraction.
        oneN_col = const.tile([P, 1], F16, tag="oneN_col", name="oneN_col")
        nc.vector.memset(oneN_col[:], 1.0 / DIN)
        ones_row = const.tile([1, P], F16, tag="ones_row", name="ones_row")
        nc.vector.memset(ones_row[:], 1.0)
        ident = const.tile([P, P], F16, tag="ident", name="ident")
        nc.sync.dma_start(ident[:], d["ident"][:])
        eps = const.tile([1, 1], F32, tag="eps", name="eps")
        nc.vector.memset(eps[:], 1e-5)

        # ---- persistent activations (fp16 [P, L] = 8KB/partition each) ----
        main = ctx.enter_context(tc.tile_pool(name="main", bufs=1))
        xs1 = [main.tile([P, L], F16, tag=f"xs1{i}", name=f"xs1{i}")
               for i in range(DH)]
        z_sb = [main.tile([P, L], F16, tag=f"z{i}", name=f"z{i}") for i in range(DH)]
        y0_sb = [main.tile([P, L], F16, tag=f"y0_{i}", name=f"y0_{i}")
                 for i in range(DH)]
        ysum = [main.tile([P, L], F16, tag=f"ys{i}", name=f"ys{i}")
                for i in range(DH)]
        xT = main.tile([P, L], F16, tag="xT", name="xT")
        delta1 = [main.tile([P, L], F16, tag=f"delta1{i}", name=f"delta1{i}")
                  for i in range(DH)]
        dU1 = [main.tile([P, L], F16, tag=f"dU1{i}", name=f"dU1{i}")
               for i in range(DH)]
        states = [main.tile([P, DST], F16, tag=f"st{i}", name=f"st{i}")
                  for i in range(DH)]
        # k=0 activations live only through P4(k=0); pool closed there.
        k0_stack = ExitStack()
        k0pool = k0_stack.enter_context(tc.tile_pool(name="k0p", bufs=1))
        xs0 = [k0pool.tile([P, L], F16, tag=f"xs0{i}", name=f"xs0{i}")
               for i in range(DH)]
        delta0 = [k0pool.tile([P, L], F16, tag=f"delta0{i}", name=f"delta0{i}")
                  for i in range(DH)]
        dU0 = [k0pool.tile([P, L], F16, tag=f"dU0{i}", name=f"dU0{i}")
               for i in range(DH)]
        xs = [xs0, xs1]
        delta = [delta0, delta1]
        dU = [dU0, dU1]

        # ========== Phase 1: in_proj -> conv+silu -> scan orderings ==========
        # Only the xh half runs here (startup critical path); the z half is a
        # deferred unit sprinkled into the k=0 scan.
        with tc.tile_pool(name="p1ps", bufs=4, space=bass.MemorySpace.PSUM) as p1ps:
            nc.sync.dma_start(xT[:], d["xT"][:])
            for po in range(DH):
                for c in range(NQ):
                    cs = slice(c * LC, (c + 1) * LC)
                    pt = p1ps.tile([P, LC], F32, tag="mm", name="mm")
                    for hb in range(LC // MM_F):
                        ps_ = slice(hb * MM_F, (hb + 1) * MM_F)
                        gs = slice(c * LC + hb * MM_F, c * LC + (hb + 1) * MM_F)
                        nc.tensor.matmul(pt[:, ps_], w_in[:, po * P:(po + 1) * P],
                                         xT[:, gs], start=True, stop=True)
                    # xh = silu(xz*conv_w + conv_b): scalar writes only the
                    # reversed (t h w) copy; the (h w t) ordering for k=0 is
                    # derived on the (startup-idle) DVE below.
                    rev = xs[1][po][:, ::-1]
                    nc.scalar.activation(rev[:, cs], pt[:], AF.Silu,
                                         bias=conv_bi[po][:], scale=conv_sc[po][:])
                src = xs[1][po][:, ::-1].rearrange("p (t hw) -> p hw t",
                                                   t=T, hw=H * W)
                dst = xs[0][po][:].rearrange("p (hw t) -> p hw t",
                                             hw=H * W, t=T)
                nc.vector.tensor_copy(dst, src)

        # ---- Phase 3 for k=0: serial (the scan needs it immediately) ----
        with tc.tile_pool(name="kd0", bufs=1) as kdpool, \
             tc.tile_pool(name="kps0", bufs=2,
                          space=bass.MemorySpace.PSUM) as kps:
            xdbl = kdpool.tile([DTR, L], F16, tag="xdbl", name="xdbl")
            for c in range(NMM):
                cs = slice(c * MM_F, (c + 1) * MM_F)
                ptA = kps.tile([DTR, MM_F], F32, tag="mmA", name="mmA")
                nc.tensor.matmul(ptA[:], w_xproj[0][0][:, 0:DTR],
                                 xs[0][0][:, cs], start=True, stop=False)
                nc.tensor.matmul(ptA[:], w_xproj[0][1][:, 0:DTR],
                                 xs[0][1][:, cs], start=False, stop=True)
                nc.vector.tensor_copy(xdbl[:, cs], ptA[:])
                ptB = kps.tile([2 * DST, MM_F], F32, tag="mmB", name="mmB")
                nc.tensor.matmul(ptB[:], w_xproj[0][0][:, DTR:40],
                                 xs[0][0][:, cs], start=True, stop=False)
                nc.tensor.matmul(ptB[:], w_xproj[0][1][:, DTR:40],
                                 xs[0][1][:, cs], start=False, stop=True)
                bcs = kdpool.tile([2 * DST, MM_F], F16, tag="bcs",
                                  name="bcs", bufs=3)
                nc.vector.tensor_copy(bcs[:], ptB[:])
                nc.sync.dma_start(d["bc_dram"][0, :, cs], bcs[:])
            # softplus(x + b) = ln(1 + exp(x + b)); exp lands in delta, ln
            # runs in place. Only quarter 0's ln/dU runs here; the rest are
            # emitted at each scan-quarter top (k0_lndu) off the prefix path.
            for i in range(DH):
                for c in range(NMM):
                    cs = slice(c * MM_F, (c + 1) * MM_F)
                    pt2 = kps.tile([P, MM_F], F32, tag="mmd", name="mmd")
                    nc.tensor.matmul(pt2[:], w_dt[0][:, i * P:(i + 1) * P],
                                     xdbl[:, cs], start=True, stop=True)
                    nc.scalar.activation(delta[0][i][:, cs], pt2[:], AF.Exp,
                                         bias=dt_bias[0][i][:])

        def k0_lndu(qq):
            sl_ = slice(qq * LC, (qq + 1) * LC)
            for i in range(DH):
                nc.scalar.activation(delta[0][i][:, sl_], delta[0][i][:, sl_],
                                     AF.Ln, bias=1.0)
            for i in range(DH):
                nc.vector.tensor_tensor(dU[0][i][:, sl_], delta[0][i][:, sl_],
                                        xs[0][i][:, sl_], ALU.mult)

        k0_lndu(0)

        # ---- Phase 3 for k=1: deferred units, sprinkled into P4(k=0) ----
        kd1_stack = ExitStack()
        kdpool1 = kd1_stack.enter_context(tc.tile_pool(name="kd1", bufs=1))
        kps1 = kd1_stack.enter_context(tc.tile_pool(name="kps1", bufs=1,
                                                    space=bass.MemorySpace.PSUM))
        xdbl1 = kdpool1.tile([DTR, L], F16, tag="xdbl", name="xdbl1")
        units = []

        def _u_xdbl(c):
            def f():
                cs = slice(c * MM_F, (c + 1) * MM_F)
                ptA = kps1.tile([DTR, MM_F], F32, tag="mmA", name="mmA1")
                nc.tensor.matmul(ptA[:], w_xproj[1][0][:, 0:DTR],
                                 xs[1][0][:, cs], start=True, stop=False)
                nc.tensor.matmul(ptA[:], w_xproj[1][1][:, 0:DTR],
                                 xs[1][1][:, cs], start=False, stop=True)
                nc.scalar.activation(xdbl1[:, cs], ptA[:], AF.Copy)
                ptB = kps1.tile([2 * DST, MM_F], F32, tag="mmB", name="mmB1")
                nc.tensor.matmul(ptB[:], w_xproj[1][0][:, DTR:40],
                                 xs[1][0][:, cs], start=True, stop=False)
                nc.tensor.matmul(ptB[:], w_xproj[1][1][:, DTR:40],
                                 xs[1][1][:, cs], start=False, stop=True)
                bcs = kdpool1.tile([2 * DST, MM_F], F16, tag="bcs",
                                   name="bcs1", bufs=2)
                nc.scalar.activation(bcs[:], ptB[:], AF.Copy)
                nc.sync.dma_start(d["bc_dram"][1, :, cs], bcs[:])
            return f

        def _u_dts(i, c):
            def f():
                cs = slice(c * MM_F, (c + 1) * MM_F)
                pt2 = kps1.tile([P, MM_F], F32, tag="mmd", name="mmd1")
                nc.tensor.matmul(pt2[:], w_dt[1][:, i * P:(i + 1) * P],
                                 xdbl1[:, cs], start=True, stop=True)
                nc.scalar.activation(delta[1][i][:, cs], pt2[:], AF.Exp,
                                     bias=dt_bias[1][i][:])
            return f

        def _u_ln():
            def f():
                for i in range(DH):
                    nc.scalar.activation(delta[1][i][:], delta[1][i][:], AF.Ln,
                                         bias=1.0)
                for i in range(DH):
                    nc.vector.tensor_tensor(dU[1][i][:], delta[1][i][:],
                                            xs[1][i][:], ALU.mult)
            return f

        def _u_z():
            def f():
                for j in range(2 * NMM):
                    po, c = DH + j // NMM, j % NMM
                    cs = slice(c * MM_F, (c + 1) * MM_F)
                    pt = kps1.tile([P, MM_F], F32, tag="mmd", name="mmz")
                    nc.tensor.matmul(pt[:], w_in[:, po * P:(po + 1) * P],
                                     xT[:, cs], start=True, stop=True)
                    nc.scalar.activation(z_sb[po - DH][:, cs], pt[:], AF.Silu)
            return f

        units.extend(_u_xdbl(c) for c in range(NMM))
        units.extend(_u_dts(i, c) for i in range(DH) for c in range(NMM))
        units.append(_u_ln())
        units.append(_u_z())

        # ---- Phase 4 (both directions): selective scan (q -> n -> i) ----
        # Pools are managed per direction so the k=0-only pools can retire in
        # LIFO order at the k boundary (freeing SBUF/PSUM for phase 5).
        fin_stack = ExitStack()
        p4_stack = ExitStack()
        work = p4_stack.enter_context(tc.tile_pool(name="sc0", bufs=2))
        bcp = p4_stack.enter_context(tc.tile_pool(name="bc0", bufs=4))
        scps = p4_stack.enter_context(tc.tile_pool(name="scps0", bufs=1,
                                                   space=bass.MemorySpace.PSUM))
        if True:
            for k in range(KG):
                    if k == 1:
                        # drain leftover k=1-prep units, retire k=0 pools,
                        # set up the sprinkled LN/gate/out_proj (phase 5).
                        while units:
                            units.pop(0)()
                        p4_stack.close()
                        kd1_stack.close()
                        k0_stack.close()
                        fin = fin_stack.enter_context(
                            tc.tile_pool(name="fin", bufs=1))
                        fch = fin_stack.enter_context(
                            tc.tile_pool(name="fch", bufs=2))
                        fps = fin_stack.enter_context(
                            tc.tile_pool(name="fps", bufs=2,
                                         space=bass.MemorySpace.PSUM))
                        mu16 = fin.tile([1, L], F16, tag="mu16", name="mu16")
                        m2_16 = fin.tile([1, L], F16, tag="m2_16", name="m2_16")
                        sq16 = fin.tile([1, L], F16, tag="sq16", name="sq16")
                        rs16 = fin.tile([1, L], F16, tag="rs16", name="rs16")

                        def _u_stats(c):
                            def f():
                                cs = slice(c * MM_F, (c + 1) * MM_F)
                                pmu = fps.tile([1, MM_F], F32, tag="pstat",
                                               name="pmu")
                                nc.tensor.matmul(pmu[:], oneN_col[:],
                                                 ysum[0][:, cs],
                                                 start=True, stop=False)
                                nc.tensor.matmul(pmu[:], oneN_col[:],
                                                 ysum[1][:, cs],
                                                 start=False, stop=True)
                                nc.scalar.activation(mu16[:, cs], pmu[:],
                                                     AF.Copy)
                                psq = fps.tile([1, MM_F], F32, tag="pstat",
                                               name="psq")
                                for i in range(DH):
                                    ysq = fch.tile([P, MM_F], F16, tag="ysq",
                                                   name="ysq")
                                    nc.vector.tensor_tensor(
                                        ysq[:], ysum[i][:, cs], ysum[i][:, cs],
                                        ALU.mult)
                                    nc.tensor.matmul(psq[:], oneN_col[:], ysq[:],
                                                     start=(i == 0),
                                                     stop=(i == DH - 1))
                                nc.scalar.activation(m2_16[:, cs], psq[:],
                                                     AF.Copy)
                            return f

                        def _u_rstd(Q):
                            def f():
                                nq = slice(Q * LC, (Q + 1) * LC)
                                nc.vector.tensor_tensor(sq16[:, nq], mu16[:, nq],
                                                        mu16[:, nq], ALU.mult)
                                nc.vector.tensor_tensor(m2_16[:, nq],
                                                        m2_16[:, nq],
                                                        sq16[:, nq],
                                                        ALU.subtract)
                                nc.scalar.activation(sq16[:, nq], m2_16[:, nq],
                                                     AF.Ln, bias=eps[:1, :])
                                nc.scalar.activation(rs16[:, nq], sq16[:, nq],
                                                     AF.Exp, scale=-0.5)
                            return f

                        def _u_norm(c):
                            def f():
                                cs = slice(c * MM_F, (c + 1) * MM_F)
                                pm = fps.tile([P, MM_F], F32, tag="pw",
                                              name="pw_m")
                                nc.tensor.matmul(pm[:], ones_row[:],
                                                 mu16[:, cs],
                                                 start=True, stop=True)
                                mrep = fch.tile([P, MM_F], F16, tag="mrep",
                                                name="mrep")
                                nc.scalar.activation(mrep[:], pm[:], AF.Copy)
                                pr = fps.tile([P, MM_F], F32, tag="pw",
                                              name="pw_r")
                                nc.tensor.matmul(pr[:], ones_row[:],
                                                 rs16[:, cs],
                                                 start=True, stop=True)
                                rrep = fch.tile([P, MM_F], F16, tag="rrep",
                                                name="rrep")
                                nc.scalar.activation(rrep[:], pr[:], AF.Copy)
                                g = []
                                for i in range(DH):
                                    yc = fch.tile([P, MM_F], F16, tag="yc",
                                                  name="yc")
                                    nc.vector.tensor_tensor(yc[:],
                                                            ysum[i][:, cs],
                                                            mrep[:],
                                                            ALU.subtract)
                                    yn = fch.tile([P, MM_F], F16, tag="yn",
                                                  name="yn")
                                    nc.vector.tensor_tensor(yn[:], yc[:],
                                                            rrep[:], ALU.mult)
                                    ya = fch.tile([P, MM_F], F16, tag="ya",
                                                  name="ya")
                                    nc.scalar.activation(ya[:], yn[:],
                                                         AF.Identity,
                                                         bias=lnb[i][:],
                                                         scale=lnw[i][:])
                                    gi = fch.tile([P, MM_F], F16, tag=f"g{i}",
                                                  name=f"g{i}")
                                    nc.vector.tensor_tensor(gi[:], ya[:],
                                                            z_sb[i][:, cs],
                                                            ALU.mult)
                                    g.append(gi)
                                po = fps.tile([P, MM_F], F32, tag="pw",
                                              name="pw_o")
                                nc.tensor.matmul(po[:], w_out[0][:], g[0][:],
                                                 start=True, stop=False)
                                nc.tensor.matmul(po[:], w_out[1][:], g[1][:],
                                                 start=False, stop=True)
                                osb = fch.tile([P, MM_F], F32, tag="osb",
                                               name="osb")
                                nc.scalar.activation(osb[:], po[:], AF.Copy)
                                nc.sync.dma_start(d["outT"][:, cs], osb[:])
                            return f

                        p4_stack = ExitStack()
                        work = p4_stack.enter_context(
                            tc.tile_pool(name="sc1", bufs=2))
                        bcp = p4_stack.enter_context(
                            tc.tile_pool(name="bc1", bufs=4))
                        scps = p4_stack.enter_context(
                            tc.tile_pool(name="scps1", bufs=1,
                                         space=bass.MemorySpace.PSUM))

                    for q in range(NQ):
                        if k == 0 and q + 1 < NQ:
                            k0_lndu(q + 1)
                        sl = slice(q * LC, (q + 1) * LC)
                        y_ps = [scps.tile([P, LC], F32, tag=f"yps{i}",
                                          name=f"yps{i}") for i in range(DH)]
                        for i in range(DH):
                            for hb in range(LC // MM_F):
                                ps_ = slice(hb * MM_F, (hb + 1) * MM_F)
                                gs = slice(q * LC + hb * MM_F,
                                           q * LC + (hb + 1) * MM_F)
                                nc.tensor.matmul(y_ps[i][:, ps_], ds_diag[k][i][:],
                                                 xs[k][i][:, gs],
                                                 start=True, stop=False)
                            if k == 1:
                                # fold y_fwd into the PSUM group: identity
                                # matmul over y0 read in reversed-natural
                                # (= this scan's) order, so the drain below
                                # yields the combined ysum without DVE work.
                                y0v = y0_sb[i][:].rearrange(
                                    "p (hw t) -> p t hw", hw=H * W,
                                    t=T)[:, ::-1, ::-1]
                                for hb in range(LC // MM_F):
                                    ps_ = slice(hb * MM_F, (hb + 1) * MM_F)
                                    tb = 4 * q + 2 * hb
                                    nc.tensor.matmul(y_ps[i][:, ps_], ident[:],
                                                     y0v[:, tb:tb + 2, :],
                                                     start=False, stop=False)
                        for n in range(DST):
                            brep = bcp.tile([P, LC], F16, tag="brep", name="brep")
                            nc.sync.dma_start(
                                brep[:],
                                d["bc_dram"][k, n:n + 1, sl].partition_broadcast(P))
                            crep = bcp.tile([P, LC], F16, tag="crep", name="crep")
                            nc.sync.dma_start(
                                crep[:],
                                d["bc_dram"][k, DST + n:DST + n + 1,
                                             sl].partition_broadcast(P))
                            for i in range(DH):
                                dA = work.tile([P, LC], F16, tag=f"dA{i}",
                                               name=f"dA{i}")
                                nc.scalar.activation(dA[:], delta[k][i][:, sl],
                                                     AF.Exp,
                                                     scale=a_mat[k][i][:, n:n + 1])
                                xin = work.tile([P, LC], F16, tag="xin", name="xin")
                                xeng = nc.gpsimd if XIN_ON_GPSIMD(n, i) else nc.vector
                                xeng.tensor_tensor(xin[:], dU[k][i][:, sl],
                                                   brep[:], ALU.mult)
                                h = work.tile([P, LC], F16, tag=f"h{i}",
                                              name=f"h{i}")
                                init = 0.0 if q == 0 else states[i][:, n:n + 1]
                                nc.vector.tensor_tensor_scan(
                                    h[:], dA[:], xin[:], init, ALU.mult, ALU.add)
                                if q < NQ - 1:
                                    nc.vector.tensor_copy(states[i][:, n:n + 1],
                                                          h[:, LC - 1:LC])
                                tmp = work.tile([P, LC], F16, tag="tmp", name="tmp")
                                eng = nc.gpsimd if TMP_ON_GPSIMD(n, i) else nc.vector
                                eng.tensor_tensor(tmp[:], crep[:], h[:], ALU.mult)
                                for hb in range(LC // MM_F):
                                    ps_ = slice(hb * MM_F, (hb + 1) * MM_F)
                                    nc.tensor.matmul(y_ps[i][:, ps_], ident[:],
                                                     tmp[:, ps_],
                                                     start=False,
                                                     stop=(n == DST - 1))
                            if units:
                                units.pop(0)()
                        for i in range(DH):
                            if k == 0:
                                nc.scalar.activation(y0_sb[i][:, sl], y_ps[i][:],
                                                     AF.Copy)
                            else:
                                # PSUM already holds y_rvs + y_fwd (in this
                                # scan's reversed-natural order); drain with a
                                # reversing Copy into the natural layout.
                                nsl = slice((NQ - 1 - q) * LC, (NQ - q) * LC)
                                nc.scalar.activation(ysum[i][:, nsl][:, ::-1],
                                                     y_ps[i][:], AF.Copy)
                        if k == 1:
                            # natural quarter Q is final -> queue its phase-5
                            # work for the next scan quarter's sprinkle slots
                            Q = NQ - 1 - q
                            units.append(_u_stats(2 * Q))
                            units.append(_u_stats(2 * Q + 1))
                            units.append(_u_rstd(Q))
                            units.append(_u_norm(2 * Q))
                            units.append(_u_norm(2 * Q + 1))
            while units:
                units.pop(0)()
        p4_stack.close()
        fin_stack.close()

        # ================= Phase 5-7: LN, gate, out_proj =================
        with tc.tile_pool(name="fin", bufs=1) as fin:
            # LN stats over DIN (partition reduce via PE 1/DIN-ones contraction)
            mu16 = fin.tile([1, L], F16, tag="mu16", name="mu16")
            m2_16 = fin.tile([1, L], F16, tag="m2_16", name="m2_16")
            sq16 = fin.tile([1, L], F16, tag="sq16", name="sq16")
            rs16 = fin.tile([1, L], F16, tag="rs16", name="rs16")
            with tc.tile_pool(name="fps1", bufs=4,
                              space=bass.MemorySpace.PSUM) as fps1:
                for c in range(NMM):
                    cs = slice(c * MM_F, (c + 1) * MM_F)
                    pmu = fps1.tile([1, MM_F], F32, tag="pmu", name="pmu")
                    nc.tensor.matmul(pmu[:], oneN_col[:], ysum[0][:, cs],
                                     start=True, stop=False)
                    nc.tensor.matmul(pmu[:], oneN_col[:], ysum[1][:, cs],
                                     start=False, stop=True)
                    nc.scalar.activation(mu16[:, cs], pmu[:], AF.Copy)
                    psq = fps1.tile([1, MM_F], F32, tag="psq", name="psq")
                    for i in range(DH):
                        ysq = fin.tile([P, MM_F], F16, tag="ysq", name="ysq",
                                       bufs=2)
                        nc.scalar.activation(ysq[:], ysum[i][:, cs], AF.Square)
                        nc.tensor.matmul(psq[:], oneN_col[:], ysq[:],
                                         start=(i == 0), stop=(i == DH - 1))
                    nc.scalar.activation(m2_16[:, cs], psq[:], AF.Copy)
            # var = E[y^2] - mu^2 ; rstd = exp(-0.5*ln(var + eps))
            nc.vector.tensor_tensor(sq16[:], mu16[:], mu16[:], ALU.mult)
            nc.vector.tensor_tensor(m2_16[:], m2_16[:], sq16[:], ALU.subtract)
            nc.scalar.activation(sq16[:], m2_16[:], AF.Ln, bias=eps[:1, :])
            nc.scalar.activation(rs16[:], sq16[:], AF.Exp, scale=-0.5)

            # normalize + affine + gate + out_proj, chunked over L
            with tc.tile_pool(name="fch", bufs=2) as fch, \
                 tc.tile_pool(name="fps2", bufs=2,
                              space=bass.MemorySpace.PSUM) as fps2:
                for c in range(NMM):
                    cs = slice(c * MM_F, (c + 1) * MM_F)
                    pm = fps2.tile([P, MM_F], F32, tag="pm", name="pm")
                    nc.tensor.matmul(pm[:], ones_row[:], mu16[:, cs],
                                     start=True, stop=True)
                    mrep = fch.tile([P, MM_F], F16, tag="mrep", name="mrep")
                    nc.scalar.activation(mrep[:], pm[:], AF.Copy)
                    pr = fps2.tile([P, MM_F], F32, tag="pr", name="pr")
                    nc.tensor.matmul(pr[:], ones_row[:], rs16[:, cs],
                                     start=True, stop=True)
                    rrep = fch.tile([P, MM_F], F16, tag="rrep", name="rrep")
                    nc.scalar.activation(rrep[:], pr[:], AF.Copy)
                    g = []
                    for i in range(DH):
                        yc = fch.tile([P, MM_F], F16, tag="yc", name="yc")
                        nc.vector.tensor_tensor(yc[:], ysum[i][:, cs], mrep[:],
                                                ALU.subtract)
                        yn = fch.tile([P, MM_F], F16, tag="yn", name="yn")
                        nc.vector.tensor_tensor(yn[:], yc[:], rrep[:], ALU.mult)
                        ya = fch.tile([P, MM_F], F16, tag="ya", name="ya")
                        nc.scalar.activation(ya[:], yn[:], AF.Identity,
                                             bias=lnb[i][:], scale=lnw[i][:])
                        gi = fch.tile([P, MM_F], F16, tag=f"g{i}", name=f"g{i}")
                        nc.vector.tensor_tensor(gi[:], ya[:], z_sb[i][:, cs],
                                                ALU.mult)
                        g.append(gi)
                    po = fps2.tile([P, MM_F], F32, tag="pout", name="pout")
                    nc.tensor.matmul(po[:], w_out[0][:], g[0][:],
                                     start=True, stop=False)
                    nc.tensor.matmul(po[:], w_out[1][:], g[1][:],
                                     start=False, stop=True)
                    osb = fch.tile([P, MM_F], F32, tag="osb", name="osb")
                    nc.scalar.activation(osb[:], po[:], AF.Copy)
                    nc.sync.dma_start(d["outT"][:, cs], osb[:])


_CACHE = {}


def _get_program():
    if "nc" not in _CACHE:
        nc = bacc.Bacc("TRN2", target_bir_lowering=False, debug=False,
                       num_devices=NCORES)
        d = _declare_drams(nc)
        with tile.TileContext(nc) as tc:
            _body(tc, d)
        nc.compile()
        _CACHE["nc"] = nc
    return _CACHE["nc"]


def _host_weights(inputs):
    f32 = lambda a: np.ascontiguousarray(np.asarray(a, np.float32))
    f16 = lambda a: np.ascontiguousarray(np.asarray(a, np.float32).astype(np.float16))
    in_proj_w = f32(inputs["in_proj_w"])        # (512, 128)
    x_proj_w = f32(inputs["x_proj_w"])          # (2, 40, 256)
    dt_w = f32(inputs["dt_w"])                  # (2, 256, 8)
    dt_b = f32(inputs["dt_b"])                  # (2, 256)
    A_logs = f32(inputs["A_logs"])              # (512, 16)
    Ds = f32(inputs["Ds"])                      # (512,)
    ds_diag = np.zeros((KG, DH, P, P), np.float16)
    dsr = Ds.reshape(KG, DH, P)
    for k in range(KG):
        for i in range(DH):
            np.fill_diagonal(ds_diag[k, i], dsr[k, i].astype(np.float16))
    m = {
        "w_in": f16(in_proj_w.T),                                   # (128, 512)
        "conv_sc": f32(inputs["conv_w"]).reshape(DH, P, 1),
        "conv_bi": f32(inputs["conv_b"]).reshape(DH, P, 1),
        "w_xproj": f16(x_proj_w.transpose(0, 2, 1).reshape(KG, DH, P, 40)),
        "w_dt": f16(dt_w.transpose(0, 2, 1)),                       # (2, 8, 256)
        "dt_bias": f32(dt_b).reshape(KG, DH, P, 1),
        "a_mat": f32(-np.exp(A_logs)).reshape(KG, DH, P, DST),
        "ds_diag": ds_diag,
        "lnw": f32(inputs["ln_w"]).reshape(DH, P, 1),
        "lnb": f32(inputs["ln_b"]).reshape(DH, P, 1),
        "w_out": f16(f32(inputs["out_proj_w"]).T.reshape(DH, P, P)),
        "ident": np.eye(P, dtype=np.float16),
    }
    return m


def kernel(**inputs):
    x = np.ascontiguousarray(np.asarray(inputs["x"], np.float32))   # (8,16,16,16,128)
    shared = _host_weights(inputs)
    nc = _get_program()
    in_maps = []
    for b in range(NCORES):
        m = dict(shared)
        m["xT"] = np.ascontiguousarray(x[b].reshape(L, DIM).T).astype(np.float16)
        in_maps.append(m)
    trace = bool(int(os.environ.get("BASS_PROFILE", "0")))
    res = run_bass_kernel_spmd(nc, in_maps, list(range(NCORES)), trace=trace)
    _CACHE["last_result"] = res
    outs = [r["outT"] for r in res.results]
    out = np.stack([o.T.reshape(T, H, W, DIM) for o in outs]).astype(np.float32)
    return out
